# revision 2
# baseline (speedup 1.0000x reference)
"""Trainium2 Bass kernel for a binarized 4-layer MLP (eval mode).

Reference computation (per row of x [B=16384, 784]):
  h1 = x @ sign(w1).T + b1;  s1 = sign(bn1(h1))        (clip doesn't change sign)
  h2 = s1 @ sign(w2).T + b2; s2 = sign(bn2(h2))
  h3 = s2 @ sign(w3).T + b3; y3 = clip(bn3(h3), -1, 1)
  z  = y3 @ w4.T + b4;       out = log_softmax(z)

Sharding: pure data-parallel over the batch across 8 NeuronCores
(weights replicated, no collectives).

Numerics:
  - L1: x is split on the HOST into two fp16 terms (a = fp16(x),
    b = fp16(x - a), residual <= 2^-22 |x|); the +-1 weights are exact in
    fp16, so fp16 matmuls accumulated in fp32 PSUM give fp32-class
    accuracy at 2-byte rate.  The two streams are concatenated along the
    contraction dim and tiled at 128 rows: 6 full tiles of a, 6 of b,
    plus one 32-row tile holding both 16-row tails -> 13 PE passes per
    output tile instead of 14 (two 7x112 streams).  The stationary
    operand for the b-stream tiles is the SAME SBUF w1 tile as the
    a-stream (sign(w1) repeats), so w1 is only DMA'd once.
  - L2/L3: both operands are exactly +-1/0 in fp8e4 -> DoubleRow fp8
    matmuls (2 k-tiles per pass) produce bit-exact integer sums in fp32
    PSUM.
  - BN + bias folding: bn(h + b) = A*h + C with A = g*rsqrt(v+eps),
    C = A*(b - m) + beta, applied per-partition by the Sign/Identity
    activations (fp32 internally).

Schedule notes (from NTFF trace analysis of the 490us version):
  - steady-state PE was already at ~99% of the per-pass roofline; the
    recoverable time was 24us of DMA-wait before the first matmul, one
    extra L1 pass per output tile, and the serial epilogue tail.
  - startup: the critical path to the first MM is now one x k-tile
    (128KB) + one 512-column block of w1 (128KB); w1 is DMA'd in
    (k-tile, 512-col) blocks interleaved with the group-0 x tiles.
  - a dummy-matmul burst on a zero tile warms the PE HAM clock gate
    (1.2 -> 2.4 GHz) while the startup DMAs are still in flight.
  - L2-L4 run full 512-column groups (no 256-halves); the log-softmax
    epilogue for group g-1 hides under group g, and in the last group it
    is interleaved per batch-tile behind L4.
"""

import sys

if "/opt/trn_rl_repo" not in sys.path:
    sys.path.insert(0, "/opt/trn_rl_repo")

import numpy as np

D_IN, H1, H2, H3, NCLS = 784, 3072, 1536, 768, 10
B, NCORES = 16384, 8
BC = B // NCORES          # batch rows per core
W = 512                   # batch columns per group
NG = BC // W              # groups per core
K1F = D_IN // 128         # full 128-row k-tiles per L1 stream: 6
K1R = D_IN - 128 * K1F    # tail rows per stream: 16
K1T = 2 * K1F + 1         # total L1 passes: 13
M1, M2, M3 = H1 // 128, H2 // 128, H3 // 128   # 24, 12, 6
NB1 = M1 // 4             # L1 column blocks of 512: 6
K2P, K3P = H1 // 256, H2 // 256                # DoubleRow k-pair iters: 12, 6
K4T = H3 // 128                                # 6
NCST = 2 * M1 + 2 * M2 + 2 * M3 + NCLS         # packed per-tile consts: 94
BN_EPS = 1e-5
NWARM = 32

_cached = {}


def _build(bc):
    import concourse.bacc as bacc
    import concourse.mybir as mybir
    import concourse.tile as tile

    dt = mybir.dt
    AF = mybir.ActivationFunctionType
    PM = mybir.MatmulPerfMode
    ALU = mybir.AluOpType

    ng = bc // W
    nc = bacc.Bacc("TRN2", target_bir_lowering=False, debug=False,
                   num_devices=NCORES)

    # xc rows: per group, 13 k-tiles of 128 partitions (tile 12 uses 32)
    xc = nc.declare_dram_parameter("xc", [ng * K1T * 128, W], dt.float16,
                                   isOutput=False)
    w1t = nc.declare_dram_parameter("w1t", [D_IN, H1], dt.float16,
                                    isOutput=False)
    w1l = nc.declare_dram_parameter("w1l", [2 * K1R, H1], dt.float16,
                                    isOutput=False)
    w2t = nc.declare_dram_parameter("w2t", [H1, H2], dt.float8e4,
                                    isOutput=False)
    w3t = nc.declare_dram_parameter("w3t", [H2, H3], dt.float8e4,
                                    isOutput=False)
    w4t = nc.declare_dram_parameter("w4t", [H3, NCLS], dt.bfloat16,
                                    isOutput=False)
    cst = nc.declare_dram_parameter("cst", [128, NCST], dt.float32,
                                    isOutput=False)
    wrm = nc.declare_dram_parameter("wrm", [128, W], dt.float16,
                                    isOutput=False)
    out = nc.declare_dram_parameter("out", [bc, NCLS], dt.float32,
                                    isOutput=True)

    with tile.TileContext(nc) as tc, \
            tc.tile_pool(name="wts", bufs=1) as wp, \
            tc.tile_pool(name="xi0", bufs=1) as xp0, \
            tc.tile_pool(name="xin", bufs=2) as xp, \
            tc.tile_pool(name="act", bufs=2) as ap_, \
            tc.tile_pool(name="eps", bufs=2) as ep, \
            tc.tile_pool(name="ps", bufs=4, space="PSUM") as ps, \
            tc.tile_pool(name="ps4", bufs=2, space="PSUM") as ps4, \
            tc.tile_pool(name="psw", bufs=1, space="PSUM") as pw:

        # ---- HAM warm-up: burn the PE clock gate from 4/8 to 8/8 while
        # the startup DMAs stream.  Reads a 128KB zero tile (first DMA in
        # the ring), writes a dedicated PSUM bank that is never read.
        wrmb = wp.tile([128, W], dt.float16, tag="wrm")
        nc.sync.dma_start(wrmb[:], wrm[:])
        pwt = pw.tile([128, W], dt.float32, tag="pw")
        for _ in range(NWARM):
            nc.tensor.matmul(pwt[:], wrmb[:, 0:128], wrmb[:],
                             start=True, stop=True)

        # ---- group-0 x tiles and the first 512-col block of w1,
        # interleaved so MM (j, mt<4) unblocks as soon as its own pair of
        # 128KB transfers lands.
        w1b = [[None] * NB1 for _ in range(K1F)]   # [kt][colblock]
        twb = [None] * NB1                         # 32-row tail tiles
        x0 = []
        for j in range(K1F):
            xj = xp0.tile([128, W], dt.float16, tag=f"x0_{j}",
                          name=f"x0_{j}")
            nc.sync.dma_start(xj[:], xc[j * 128:(j + 1) * 128, :])
            x0.append(xj)
            wkb = wp.tile([128, W], dt.float16, tag=f"w1_{j}_0",
                          name=f"w1_{j}_0")
            nc.sync.dma_start(wkb[:], w1t[j * 128:(j + 1) * 128, 0:W])
            w1b[j][0] = wkb
        for j in range(K1F, K1T):
            p = 128 if j < 2 * K1F else 2 * K1R
            xj = xp0.tile([p, W], dt.float16, tag=f"x0_{j}", name=f"x0_{j}")
            nc.sync.dma_start(xj[:], xc[j * 128:j * 128 + p, :])
            x0.append(xj)
        twb[0] = wp.tile([2 * K1R, W], dt.float16, tag="tw0", name="tw0")
        nc.sync.dma_start(twb[0][:], w1l[:, 0:W])
        cstb = wp.tile([128, NCST], dt.float32, tag="cst")
        nc.sync.dma_start(cstb[:], cst[:])

        # const views into the packed per-output-tile scale/bias table
        def a1v(mt): return cstb[:, mt:mt + 1]
        def c1v(mt): return cstb[:, M1 + mt:M1 + mt + 1]
        def a2v(mt): return cstb[:, 2 * M1 + mt:2 * M1 + mt + 1]
        def c2v(mt): return cstb[:, 2 * M1 + M2 + mt:2 * M1 + M2 + mt + 1]
        def a3v(mt):
            o = 2 * M1 + 2 * M2
            return cstb[:, o + mt:o + mt + 1]
        def c3v(mt):
            o = 2 * M1 + 2 * M2 + M3
            return cstb[:, o + mt:o + mt + 1]
        b4v = cstb[:, NCST - NCLS:NCST]

        # ---- remaining w1 column blocks, then w2/w3/w4; ring order
        # keeps every transfer ahead of its first consumer.
        for b in range(1, NB1):
            for j in range(K1F):
                wkb = wp.tile([128, W], dt.float16, tag=f"w1_{j}_{b}",
                              name=f"w1_{j}_{b}")
                nc.sync.dma_start(wkb[:],
                                  w1t[j * 128:(j + 1) * 128,
                                      b * W:(b + 1) * W])
                w1b[j][b] = wkb
            twb[b] = wp.tile([2 * K1R, W], dt.float16, tag=f"tw{b}",
                             name=f"tw{b}")
            nc.sync.dma_start(twb[b][:], w1l[:, b * W:(b + 1) * W])

        w2sb = wp.tile([128, 2 * K2P, H2], dt.float8e4, tag="w2")
        for kt in range(2 * K2P):
            nc.sync.dma_start(w2sb[:, kt, :],
                              w2t[kt * 128:(kt + 1) * 128, :])
        w3sb = wp.tile([128, 2 * K3P, H3], dt.float8e4, tag="w3")
        for kt in range(2 * K3P):
            nc.sync.dma_start(w3sb[:, kt, :],
                              w3t[kt * 128:(kt + 1) * 128, :])
        w4sb = wp.tile([128, K4T, NCLS], dt.bfloat16, tag="w4")
        nc.sync.dma_start(w4sb[:],
                          w4t.ap().rearrange("(kt p) n -> p kt n", p=128))

        zout = wp.tile([128, ng * 4, NCLS], dt.float32, tag="zout")
        ssum = wp.tile([128, ng * 4], dt.float32, tag="ssum")
        lsum = wp.tile([128, ng * 4], dt.float32, tag="lsum")

        def l1_lhs(j, b):
            # stationary for concat k-tile j, output cols [512b, 512b+512):
            # the xb-stream tiles reuse the xa-stream's w1 tiles.
            if j < K1F:
                return w1b[j][b]
            if j < 2 * K1F:
                return w1b[j - K1F][b]
            return twb[b]

        def emit_epilogue(lo, hi, batch_dma=True):
            # log_softmax over the free dim; |z| is small so no max-shift
            for r in range(lo, hi):
                e = ep.tile([128, NCLS], dt.float32, tag="e")
                nc.scalar.activation(e[:], zout[:, r, :], AF.Exp,
                                     accum_out=ssum[:, r:r + 1])
            nc.scalar.activation(lsum[:, lo:hi], ssum[:, lo:hi], AF.Ln)
            for r in range(lo, hi):
                nc.vector.tensor_scalar(zout[:, r, :], zout[:, r, :],
                                        lsum[:, r:r + 1], None,
                                        op0=ALU.subtract)
            if batch_dma:
                nc.sync.dma_start(
                    out.ap()[lo * 128:hi * 128, :].rearrange(
                        "(g p) n -> p g n", p=128),
                    zout[:, lo:hi, :])

        for g in range(ng):
            if g == 0:
                xtiles = x0
            else:
                xgt = xp.tile([128, 2 * K1F, W], dt.float16, tag="xin")
                r0 = g * K1T * 128
                nc.sync.dma_start(
                    xgt[:], xc[r0:r0 + 2 * K1F * 128, :].rearrange(
                        "(t p) w -> p t w", p=128))
                xtl = xp.tile([2 * K1R, W], dt.float16, tag="xtl")
                nc.sync.dma_start(
                    xtl[:], xc[r0 + 2 * K1F * 128:
                               r0 + 2 * K1F * 128 + 2 * K1R, :])
                xtiles = [xgt[:, t, :] for t in range(2 * K1F)] + [xtl[:]]
                # epilogue for the previous group hides under this group's
                # L1 matmuls (issued after the x DMA so the in-order DMA
                # ring never parks a not-yet-ready out transfer ahead of
                # a prefetch)
                emit_epilogue(4 * (g - 1), 4 * g)

            # ---- L1: [784 -> 3072], concat fp16 streams, 13 passes
            h1sb = ap_.tile([128, 2 * K2P, W], dt.float8e4, tag="h1")
            if g == 0:
                # k-outer over blocks of 4 PSUM banks: the PE consumes
                # each (x, w1) k-tile pair as its DMA lands instead of
                # idling until the whole stream arrives.
                for b in range(NB1):
                    pts = [ps.tile([128, W], dt.float32, tag="ps",
                                   name=f"pt{i}") for i in range(4)]
                    for j in range(K1T):
                        lhs = l1_lhs(j, b)
                        for i in range(4):
                            nc.tensor.matmul(pts[i][:],
                                             lhs[:, i * 128:(i + 1) * 128],
                                             xtiles[j][:],
                                             start=(j == 0),
                                             stop=(j == K1T - 1))
                    for i in range(4):
                        mt = 4 * b + i
                        nc.scalar.activation(h1sb[:, mt, :], pts[i][:],
                                             AF.Sign, bias=c1v(mt),
                                             scale=a1v(mt))
            else:
                for mt in range(M1):
                    b, i = mt // 4, mt % 4
                    pt = ps.tile([128, W], dt.float32, tag="ps")
                    for j in range(K1T):
                        nc.tensor.matmul(pt[:],
                                         l1_lhs(j, b)[:, i * 128:(i + 1) * 128],
                                         xtiles[j][:],
                                         start=(j == 0), stop=(j == K1T - 1))
                    nc.scalar.activation(h1sb[:, mt, :], pt[:], AF.Sign,
                                         bias=c1v(mt), scale=a1v(mt))

            # ---- L2: [3072 -> 1536], fp8 DoubleRow, full 512-col group
            h2sb = ap_.tile([128, 2 * K3P, W], dt.float8e4, tag="h2")
            for mt in range(M2):
                pt = ps.tile([128, W], dt.float32, tag="ps")
                for kp in range(K2P):
                    nc.tensor.matmul(
                        pt[:],
                        w2sb[:, 2 * kp:2 * kp + 2, mt * 128:(mt + 1) * 128],
                        h1sb[:, 2 * kp:2 * kp + 2, :],
                        start=(kp == 0), stop=(kp == K2P - 1),
                        perf_mode=PM.DoubleRow)
                nc.scalar.activation(h2sb[:, mt, :], pt[:], AF.Sign,
                                     bias=c2v(mt), scale=a2v(mt))

            # ---- L3: [1536 -> 768], fp8 DoubleRow; scale/bias on the
            # Scalar engine (Identity), clip on DVE; bf16 output keeps L4
            # single-pass (fp32 moving data double-pumps the PE)
            h3c = ap_.tile([128, K4T, W], dt.bfloat16, tag="h3")
            for mt in range(M3):
                pt = ps.tile([128, W], dt.float32, tag="ps")
                for kp in range(K3P):
                    nc.tensor.matmul(
                        pt[:],
                        w3sb[:, 2 * kp:2 * kp + 2, mt * 128:(mt + 1) * 128],
                        h2sb[:, 2 * kp:2 * kp + 2, :],
                        start=(kp == 0), stop=(kp == K3P - 1),
                        perf_mode=PM.DoubleRow)
                nc.scalar.activation(h3c[:, mt, :], pt[:], AF.Identity,
                                     bias=c3v(mt), scale=a3v(mt))
                nc.vector.tensor_scalar(h3c[:, mt, :], h3c[:, mt, :],
                                        1.0, -1.0, op0=ALU.min, op1=ALU.max)

            # ---- L4: logits z = y3 @ w4.T + b4, [batch-tile, 10]
            for bt in range(W // 128):
                r = 4 * g + bt
                p4 = ps4.tile([128, NCLS], dt.float32, tag="p4")
                for kt in range(K4T):
                    nc.tensor.matmul(p4[:],
                                     h3c[:, kt, bt * 128:(bt + 1) * 128],
                                     w4sb[:, kt, :],
                                     start=(kt == 0), stop=(kt == K4T - 1))
                nc.vector.tensor_add(zout[:, r, :], p4[:], b4v)
                if g == ng - 1:
                    # last group: per-tile epilogue rides behind the next
                    # batch-tile's L4 matmuls; only the last tile's short
                    # chain runs after the final MM
                    emit_epilogue(r, r + 1)

    nc.finalize()
    return nc


def _prep(x, w1, b1, w2, b2, w3, b3, w4, b4,
          g1, be1, m1, v1, g2, be2, m2, v2, g3, be3, m3, v3):
    """Host-side layout prep: transposes, binarized weight casts, BN folds,
    and the fp16 split + k-concat repack of x."""
    import concourse.mybir as mybir
    f8 = mybir.dt.np(mybir.dt.float8e4)
    bf16 = mybir.dt.np(mybir.dt.bfloat16)

    def fold(g, be, m, v, b):
        a = (g / np.sqrt(v + np.float32(BN_EPS))).astype(np.float32)
        c = (a * (b - m) + be).astype(np.float32)
        return a, c

    a1, c1 = fold(g1, be1, m1, v1, b1)
    a2, c2 = fold(g2, be2, m2, v2, b2)
    a3, c3 = fold(g3, be3, m3, v3, b3)

    def cols(v, mtiles):
        return v.reshape(mtiles, 128).T

    cstm = np.zeros((128, NCST), np.float32)
    o = 0
    for v, m in ((a1, M1), (c1, M1), (a2, M2), (c2, M2), (a3, M3), (c3, M3)):
        cstm[:, o:o + m] = cols(v, m)
        o += m
    cstm[:, o:o + NCLS] = b4.astype(np.float32)[None, :]

    s1t = np.sign(w1).T.astype(np.float16)         # [784, 3072]
    pre = dict(
        w1t=np.ascontiguousarray(s1t),
        w1l=np.ascontiguousarray(np.concatenate([s1t[128 * K1F:],
                                                 s1t[128 * K1F:]], axis=0)),
        w2t=np.ascontiguousarray(np.sign(w2).T).astype(f8),
        w3t=np.ascontiguousarray(np.sign(w3).T).astype(f8),
        w4t=np.ascontiguousarray(w4.T).astype(bf16),
        cst=cstm,
        wrm=np.zeros((128, W), np.float16),
    )

    # fp16 split of x, transposed and repacked as [core][group][13 k-tiles]
    xa = x.astype(np.float16)
    xb = (x.astype(np.float32) - xa.astype(np.float32)).astype(np.float16)
    xat = xa.T.reshape(D_IN, NCORES, NG, W)
    xbt = xb.T.reshape(D_IN, NCORES, NG, W)
    pk = np.zeros((NCORES, NG, K1T, 128, W), np.float16)
    for j in range(K1F):
        pk[:, :, j] = xat[j * 128:(j + 1) * 128].transpose(1, 2, 0, 3)
        pk[:, :, K1F + j] = xbt[j * 128:(j + 1) * 128].transpose(1, 2, 0, 3)
    pk[:, :, 2 * K1F, :K1R] = xat[128 * K1F:].transpose(1, 2, 0, 3)
    pk[:, :, 2 * K1F, K1R:2 * K1R] = xbt[128 * K1F:].transpose(1, 2, 0, 3)
    return pre, pk


def run(inputs, **spmd_kwargs):
    from concourse.bass_utils import run_bass_kernel_spmd

    if "nc" not in _cached:
        _cached["nc"] = _build(BC)
    nc = _cached["nc"]

    inputs = {k: np.asarray(v) for k, v in inputs.items()}
    pre, pk = _prep(**inputs)

    in_maps = []
    for core in range(NCORES):
        m = dict(pre)
        m["xc"] = np.ascontiguousarray(pk[core].reshape(NG * K1T * 128, W))
        in_maps.append(m)

    res = run_bass_kernel_spmd(nc, in_maps, list(range(NCORES)), **spmd_kwargs)
    outs = [res.results[i]["out"] for i in range(NCORES)]
    return res, np.concatenate(outs, axis=0).astype(np.float32)


def kernel(**inputs):
    return run(inputs)[1]


# revision 9
# speedup vs baseline: 1.2040x; 1.2040x over previous
"""Trainium2 Bass kernel for a binarized 4-layer MLP (eval mode).

Reference computation (per row of x [B=16384, 784]):
  h1 = x @ sign(w1).T + b1;  s1 = sign(bn1(h1))        (clip doesn't change sign)
  h2 = s1 @ sign(w2).T + b2; s2 = sign(bn2(h2))
  h3 = s2 @ sign(w3).T + b3; y3 = clip(bn3(h3), -1, 1)
  z  = y3 @ w4.T + b4;       out = log_softmax(z)

Sharding: pure data-parallel over the batch across 8 NeuronCores
(weights replicated, no collectives).

Numerics:
  - L1: x is split on the HOST into two fp16 terms (a = fp16(x),
    b = fp16(x - a), residual <= 2^-22 |x|); the +-1 weights are exact in
    fp16, so fp16 matmuls accumulated in fp32 PSUM give fp32-class
    accuracy at 2-byte rate.  The two streams are concatenated along the
    contraction dim and tiled at 128 rows: 6 full tiles of a, 6 of b,
    plus one 32-row tile holding both 16-row tails -> 13 PE passes per
    output tile instead of 14 (two 7x112 streams).  The stationary
    operand for the b-stream tiles is the SAME SBUF w1 tile as the
    a-stream (sign(w1) repeats), so w1 is only DMA'd once.
  - L2/L3: both operands are exactly +-1/0 in fp8e4 -> DoubleRow fp8
    matmuls (2 k-tiles per pass) produce bit-exact integer sums in fp32
    PSUM.
  - BN + bias folding: bn(h + b) = A*h + C with A = g*rsqrt(v+eps),
    C = A*(b - m) + beta, applied per-partition by the Sign/Identity
    activations (fp32 internally).

Schedule notes (from NTFF trace analysis of the 490us version):
  - steady-state PE was already at ~99% of the per-pass roofline; the
    recoverable time was 24us of DMA-wait before the first matmul, one
    extra L1 pass per output tile, and the serial epilogue tail.
  - startup: the critical path to the first MM is now one x k-tile
    (128KB) + one 512-column block of w1 (128KB); w1 is DMA'd in
    (k-tile, 512-col) blocks interleaved with the group-0 x tiles.
  - a dummy-matmul burst on a zero tile warms the PE HAM clock gate
    (1.2 -> 2.4 GHz) while the startup DMAs are still in flight.
  - L2-L4 run full 512-column groups (no 256-halves); the log-softmax
    epilogue for group g-1 hides under group g, and in the last group it
    is interleaved per batch-tile behind L4.
"""

import sys

if "/opt/trn_rl_repo" not in sys.path:
    sys.path.insert(0, "/opt/trn_rl_repo")

import numpy as np

D_IN, H1, H2, H3, NCLS = 784, 3072, 1536, 768, 10
B, NCORES = 16384, 8
BC = B // NCORES          # batch rows per core
W = 512                   # batch columns per group
NG = BC // W              # groups per core
K1F = D_IN // 128         # full 128-row k-tiles per L1 stream: 6
K1R = D_IN - 128 * K1F    # tail rows per stream: 16
K1T = 2 * K1F + 1         # total L1 passes: 13
M1, M2, M3 = H1 // 128, H2 // 128, H3 // 128   # 24, 12, 6
NB1 = M1 // 4             # L1 column blocks of 512: 6
K2P, K3P = H1 // 256, H2 // 256                # DoubleRow k-pair iters: 12, 6
K4T = H3 // 128                                # 6
NCST = 2 * M1 + 2 * M2 + 2 * M3 + NCLS         # packed per-tile consts: 94
BN_EPS = 1e-5
NWARM = 32

_cached = {}


def _build(bc):
    import concourse.bacc as bacc
    import concourse.mybir as mybir
    import concourse.tile as tile

    dt = mybir.dt
    AF = mybir.ActivationFunctionType
    PM = mybir.MatmulPerfMode
    ALU = mybir.AluOpType

    ng = bc // W
    nc = bacc.Bacc("TRN2", target_bir_lowering=False, debug=False,
                   num_devices=NCORES)

    # xc rows: per group, 13 k-tiles of 128 partitions (tile 12 uses 32)
    xc = nc.declare_dram_parameter("xc", [ng * K1T * 128, W], dt.float16,
                                   isOutput=False)
    w1t = nc.declare_dram_parameter("w1t", [D_IN, H1], dt.float16,
                                    isOutput=False)
    w1l = nc.declare_dram_parameter("w1l", [2 * K1R, H1], dt.float16,
                                    isOutput=False)
    w2t = nc.declare_dram_parameter("w2t", [H1, H2], dt.float8e4,
                                    isOutput=False)
    w3t = nc.declare_dram_parameter("w3t", [H2, H3], dt.float8e4,
                                    isOutput=False)
    w4t = nc.declare_dram_parameter("w4t", [H3, NCLS], dt.bfloat16,
                                    isOutput=False)
    cst = nc.declare_dram_parameter("cst", [128, NCST], dt.float32,
                                    isOutput=False)
    wrm = nc.declare_dram_parameter("wrm", [128, W], dt.float16,
                                    isOutput=False)
    out = nc.declare_dram_parameter("out", [bc, NCLS], dt.float32,
                                    isOutput=True)

    with tile.TileContext(nc) as tc, \
            tc.tile_pool(name="wts", bufs=1) as wp, \
            tc.tile_pool(name="xi0", bufs=1) as xp0, \
            tc.tile_pool(name="xin", bufs=2) as xp, \
            tc.tile_pool(name="act", bufs=2) as ap_, \
            tc.tile_pool(name="eps", bufs=2) as ep, \
            tc.tile_pool(name="ps", bufs=4, space="PSUM") as ps, \
            tc.tile_pool(name="ps4", bufs=2, space="PSUM") as ps4, \
            tc.tile_pool(name="psw", bufs=1, space="PSUM") as pw:

        # ---- HAM warm-up: burn the PE clock gate from 4/8 to 8/8 while
        # the startup DMAs stream.  Reads a 128KB zero tile (first DMA in
        # the ring), writes a dedicated PSUM bank that is never read.
        wrmb = wp.tile([128, W], dt.float16, tag="wrm")
        nc.sync.dma_start(wrmb[:], wrm[:])
        pwt = pw.tile([128, W], dt.float32, tag="pw")
        for _ in range(NWARM):
            nc.tensor.matmul(pwt[:], wrmb[:, 0:128], wrmb[:],
                             start=True, stop=True)

        # ---- group-0 x tiles and the first 512-col block of w1,
        # interleaved so MM (j, mt<4) unblocks as soon as its own pair of
        # 128KB transfers lands.
        w1b = [[None] * NB1 for _ in range(K1F)]   # [kt][colblock]
        twb = [None] * NB1                         # 32-row tail tiles
        x0 = [None] * K1T
        for k in range(K1F):
            for j in (k, k + K1F):                 # xa tile + its xb partner
                xj = xp0.tile([128, W], dt.float16, tag=f"x0_{j}",
                              name=f"x0_{j}")
                nc.sync.dma_start(xj[:], xc[j * 128:(j + 1) * 128, :])
                x0[j] = xj
            wkb = wp.tile([128, W], dt.float16, tag=f"w1_{k}_0",
                          name=f"w1_{k}_0")
            nc.sync.dma_start(wkb[:], w1t[k * 128:(k + 1) * 128, 0:W])
            w1b[k][0] = wkb
        xj = xp0.tile([2 * K1R, W], dt.float16, tag="x0_12", name="x0_12")
        nc.sync.dma_start(xj[:], xc[2 * K1F * 128:2 * K1F * 128 + 2 * K1R, :])
        x0[2 * K1F] = xj
        twb[0] = wp.tile([2 * K1R, W], dt.float16, tag="tw0", name="tw0")
        nc.sync.dma_start(twb[0][:], w1l[:, 0:W])
        cstb = wp.tile([128, NCST], dt.float32, tag="cst")
        nc.sync.dma_start(cstb[:], cst[:])

        # const views into the packed per-output-tile scale/bias table
        def a1v(mt): return cstb[:, mt:mt + 1]
        def c1v(mt): return cstb[:, M1 + mt:M1 + mt + 1]
        def a2v(mt): return cstb[:, 2 * M1 + mt:2 * M1 + mt + 1]
        def c2v(mt): return cstb[:, 2 * M1 + M2 + mt:2 * M1 + M2 + mt + 1]
        def a3v(mt):
            o = 2 * M1 + 2 * M2
            return cstb[:, o + mt:o + mt + 1]
        def c3v(mt):
            o = 2 * M1 + 2 * M2 + M3
            return cstb[:, o + mt:o + mt + 1]
        b4v = cstb[:, NCST - NCLS:NCST]

        # ---- remaining w1 column blocks, then w2/w3/w4; ring order
        # keeps every transfer ahead of its first consumer.
        for b in range(1, NB1):
            for j in range(K1F):
                wkb = wp.tile([128, W], dt.float16, tag=f"w1_{j}_{b}",
                              name=f"w1_{j}_{b}")
                nc.sync.dma_start(wkb[:],
                                  w1t[j * 128:(j + 1) * 128,
                                      b * W:(b + 1) * W])
                w1b[j][b] = wkb
            twb[b] = wp.tile([2 * K1R, W], dt.float16, tag=f"tw{b}",
                             name=f"tw{b}")
            nc.sync.dma_start(twb[b][:], w1l[:, b * W:(b + 1) * W])

        w2sb = wp.tile([128, 2 * K2P, H2], dt.float8e4, tag="w2")
        for kt in range(2 * K2P):
            nc.sync.dma_start(w2sb[:, kt, :],
                              w2t[kt * 128:(kt + 1) * 128, :])
        w3sb = wp.tile([128, 2 * K3P, H3], dt.float8e4, tag="w3")
        for kt in range(2 * K3P):
            nc.sync.dma_start(w3sb[:, kt, :],
                              w3t[kt * 128:(kt + 1) * 128, :])
        w4sb = wp.tile([128, K4T, NCLS], dt.bfloat16, tag="w4")
        nc.sync.dma_start(w4sb[:],
                          w4t.ap().rearrange("(kt p) n -> p kt n", p=128))

        zout = wp.tile([128, ng * 4, NCLS], dt.float32, tag="zout")
        ssum = wp.tile([128, ng * 4], dt.float32, tag="ssum")
        lsum = wp.tile([128, ng * 4], dt.float32, tag="lsum")

        def l1_lhs(j, b):
            # stationary for concat k-tile j, output cols [512b, 512b+512):
            # the xb-stream tiles reuse the xa-stream's w1 tiles.
            if j < K1F:
                return w1b[j][b]
            if j < 2 * K1F:
                return w1b[j - K1F][b]
            return twb[b]

        # L1 pass order pairs the xa tile with its xb partner so consecutive
        # matmuls share the stationary operand (halves live LDWEIGHTS
        # pressure, which otherwise paces the PE above the N-cycle rate)
        J_ORDER = [j for k in range(K1F) for j in (k, k + K1F)] + [2 * K1F]

        def emit_epilogue(lo, hi, batch_dma=True):
            # log_softmax over the free dim; |z| is small so no max-shift
            for r in range(lo, hi):
                e = ep.tile([128, NCLS], dt.float32, tag="e")
                nc.scalar.activation(e[:], zout[:, r, :], AF.Exp,
                                     accum_out=ssum[:, r:r + 1])
            nc.scalar.activation(lsum[:, lo:hi], ssum[:, lo:hi], AF.Ln)
            for r in range(lo, hi):
                nc.vector.tensor_scalar(zout[:, r, :], zout[:, r, :],
                                        lsum[:, r:r + 1], None,
                                        op0=ALU.subtract)
            if batch_dma:
                nc.sync.dma_start(
                    out.ap()[lo * 128:hi * 128, :].rearrange(
                        "(g p) n -> p g n", p=128),
                    zout[:, lo:hi, :])

        for g in range(ng):
            if g == 0:
                xtiles = x0
            else:
                xgt = xp.tile([128, 2 * K1F, W], dt.float16, tag="xin")
                r0 = g * K1T * 128
                nc.sync.dma_start(
                    xgt[:], xc[r0:r0 + 2 * K1F * 128, :].rearrange(
                        "(t p) w -> p t w", p=128))
                xtl = xp.tile([2 * K1R, W], dt.float16, tag="xtl")
                nc.sync.dma_start(
                    xtl[:], xc[r0 + 2 * K1F * 128:
                               r0 + 2 * K1F * 128 + 2 * K1R, :])
                xtiles = [xgt[:, t, :] for t in range(2 * K1F)] + [xtl[:]]
                # epilogue for the previous group hides under this group's
                # L1 matmuls (issued after the x DMA so the in-order DMA
                # ring never parks a not-yet-ready out transfer ahead of
                # a prefetch)
                emit_epilogue(4 * (g - 1), 4 * g)

            # ---- L1: [784 -> 3072], concat fp16 streams, 13 passes
            h1sb = ap_.tile([128, 2 * K2P, W], dt.float8e4, tag="h1")
            if g == 0:
                # k-outer over blocks of 4 PSUM banks: the PE consumes
                # each (x, w1) k-tile pair as its DMA lands instead of
                # idling until the whole stream arrives.
                for b in range(NB1):
                    pts = [ps.tile([128, W], dt.float32, tag="ps",
                                   name=f"pt{i}") for i in range(4)]
                    for k in range(K1F):
                        lhs = w1b[k][b]
                        for i in range(4):
                            # xa then xb back-to-back: same stationary slice
                            nc.tensor.matmul(pts[i][:],
                                             lhs[:, i * 128:(i + 1) * 128],
                                             xtiles[k][:],
                                             start=(k == 0), stop=False)
                            nc.tensor.matmul(pts[i][:],
                                             lhs[:, i * 128:(i + 1) * 128],
                                             xtiles[k + K1F][:],
                                             start=False, stop=False)
                    for i in range(4):
                        nc.tensor.matmul(pts[i][:],
                                         twb[b][:, i * 128:(i + 1) * 128],
                                         xtiles[2 * K1F][:],
                                         start=False, stop=True)
                    for i in range(4):
                        mt = 4 * b + i
                        nc.scalar.activation(h1sb[:, mt, :], pts[i][:],
                                             AF.Sign, bias=c1v(mt),
                                             scale=a1v(mt))
            else:
                for mt in range(M1):
                    b, i = mt // 4, mt % 4
                    pt = ps.tile([128, W], dt.float32, tag="ps")
                    for jj, j in enumerate(J_ORDER):
                        nc.tensor.matmul(pt[:],
                                         l1_lhs(j, b)[:, i * 128:(i + 1) * 128],
                                         xtiles[j][:],
                                         start=(jj == 0), stop=(jj == K1T - 1))
                    nc.scalar.activation(h1sb[:, mt, :], pt[:], AF.Sign,
                                         bias=c1v(mt), scale=a1v(mt))

            # ---- L2-L4 iterate over 256-column halves of the group (a
            # 512-wide DoubleRow matmul paces ~0.52 ns/col on HW vs 0.425
            # at 256)
            for h in range(2):
                hs = slice(h * 256, (h + 1) * 256)
                # ---- L2: [3072 -> 1536], fp8 DoubleRow
                h2sb = ap_.tile([128, 2 * K3P, 256], dt.float8e4, tag="h2")
                for mt in range(M2):
                    pt = ps.tile([128, 256], dt.float32, tag="ps")
                    for kp in range(K2P):
                        nc.tensor.matmul(
                            pt[:],
                            w2sb[:, 2 * kp:2 * kp + 2,
                                 mt * 128:(mt + 1) * 128],
                            h1sb[:, 2 * kp:2 * kp + 2, hs],
                            start=(kp == 0), stop=(kp == K2P - 1),
                            perf_mode=PM.DoubleRow)
                    nc.scalar.activation(h2sb[:, mt, :], pt[:], AF.Sign,
                                         bias=c2v(mt), scale=a2v(mt))

                # ---- L3: [1536 -> 768], fp8 DoubleRow; scale/bias on the
                # Scalar engine (Identity), clip on DVE; bf16 output keeps
                # L4 single-pass (fp32 moving data double-pumps the PE)
                h3c = ap_.tile([128, K4T, 256], dt.bfloat16, tag="h3")
                for mt in range(M3):
                    pt = ps.tile([128, 256], dt.float32, tag="ps")
                    for kp in range(K3P):
                        nc.tensor.matmul(
                            pt[:],
                            w3sb[:, 2 * kp:2 * kp + 2,
                                 mt * 128:(mt + 1) * 128],
                            h2sb[:, 2 * kp:2 * kp + 2, :],
                            start=(kp == 0), stop=(kp == K3P - 1),
                            perf_mode=PM.DoubleRow)
                    nc.scalar.activation(h3c[:, mt, :], pt[:], AF.Identity,
                                         bias=c3v(mt), scale=a3v(mt))
                    nc.vector.tensor_scalar(h3c[:, mt, :], h3c[:, mt, :],
                                            1.0, -1.0, op0=ALU.min,
                                            op1=ALU.max)

                # ---- L4: logits z = y3 @ w4.T + b4, [batch-tile, 10]
                for bt in range(2):
                    r = 4 * g + 2 * h + bt
                    p4 = ps4.tile([128, NCLS], dt.float32, tag="p4")
                    for kt in range(K4T):
                        nc.tensor.matmul(p4[:],
                                         h3c[:, kt,
                                             bt * 128:(bt + 1) * 128],
                                         w4sb[:, kt, :],
                                         start=(kt == 0),
                                         stop=(kt == K4T - 1))
                    nc.vector.tensor_add(zout[:, r, :], p4[:], b4v)
                    if g == ng - 1:
                        # last group: per-tile epilogue rides behind the
                        # next batch-tile's L4 matmuls; only the last
                        # tile's short chain runs after the final MM
                        emit_epilogue(r, r + 1)

    nc.finalize()
    return nc


def _prep(x, w1, b1, w2, b2, w3, b3, w4, b4,
          g1, be1, m1, v1, g2, be2, m2, v2, g3, be3, m3, v3):
    """Host-side layout prep: transposes, binarized weight casts, BN folds,
    and the fp16 split + k-concat repack of x."""
    import concourse.mybir as mybir
    f8 = mybir.dt.np(mybir.dt.float8e4)
    bf16 = mybir.dt.np(mybir.dt.bfloat16)

    def fold(g, be, m, v, b):
        a = (g / np.sqrt(v + np.float32(BN_EPS))).astype(np.float32)
        c = (a * (b - m) + be).astype(np.float32)
        return a, c

    a1, c1 = fold(g1, be1, m1, v1, b1)
    a2, c2 = fold(g2, be2, m2, v2, b2)
    a3, c3 = fold(g3, be3, m3, v3, b3)

    def cols(v, mtiles):
        return v.reshape(mtiles, 128).T

    cstm = np.zeros((128, NCST), np.float32)
    o = 0
    for v, m in ((a1, M1), (c1, M1), (a2, M2), (c2, M2), (a3, M3), (c3, M3)):
        cstm[:, o:o + m] = cols(v, m)
        o += m
    cstm[:, o:o + NCLS] = b4.astype(np.float32)[None, :]

    s1t = np.sign(w1).T.astype(np.float16)         # [784, 3072]
    pre = dict(
        w1t=np.ascontiguousarray(s1t),
        w1l=np.ascontiguousarray(np.concatenate([s1t[128 * K1F:],
                                                 s1t[128 * K1F:]], axis=0)),
        w2t=np.ascontiguousarray(np.sign(w2).T).astype(f8),
        w3t=np.ascontiguousarray(np.sign(w3).T).astype(f8),
        w4t=np.ascontiguousarray(w4.T).astype(bf16),
        cst=cstm,
        wrm=np.zeros((128, W), np.float16),
    )

    # fp16 split of x, transposed and repacked as [core][group][13 k-tiles]
    xa = x.astype(np.float16)
    xb = (x.astype(np.float32) - xa.astype(np.float32)).astype(np.float16)
    xat = xa.T.reshape(D_IN, NCORES, NG, W)
    xbt = xb.T.reshape(D_IN, NCORES, NG, W)
    pk = np.zeros((NCORES, NG, K1T, 128, W), np.float16)
    for j in range(K1F):
        pk[:, :, j] = xat[j * 128:(j + 1) * 128].transpose(1, 2, 0, 3)
        pk[:, :, K1F + j] = xbt[j * 128:(j + 1) * 128].transpose(1, 2, 0, 3)
    pk[:, :, 2 * K1F, :K1R] = xat[128 * K1F:].transpose(1, 2, 0, 3)
    pk[:, :, 2 * K1F, K1R:2 * K1R] = xbt[128 * K1F:].transpose(1, 2, 0, 3)
    return pre, pk


def run(inputs, **spmd_kwargs):
    from concourse.bass_utils import run_bass_kernel_spmd

    if "nc" not in _cached:
        _cached["nc"] = _build(BC)
    nc = _cached["nc"]

    inputs = {k: np.asarray(v) for k, v in inputs.items()}
    pre, pk = _prep(**inputs)

    in_maps = []
    for core in range(NCORES):
        m = dict(pre)
        m["xc"] = np.ascontiguousarray(pk[core].reshape(NG * K1T * 128, W))
        in_maps.append(m)

    res = run_bass_kernel_spmd(nc, in_maps, list(range(NCORES)), **spmd_kwargs)
    outs = [res.results[i]["out"] for i in range(NCORES)]
    return res, np.concatenate(outs, axis=0).astype(np.float32)


def kernel(**inputs):
    return run(inputs)[1]


# revision 11
# speedup vs baseline: 1.3302x; 1.1047x over previous
"""Trainium2 Bass kernel for a binarized 4-layer MLP (eval mode).

Reference computation (per row of x [B=16384, 784]):
  h1 = x @ sign(w1).T + b1;  s1 = sign(bn1(h1))        (clip doesn't change sign)
  h2 = s1 @ sign(w2).T + b2; s2 = sign(bn2(h2))
  h3 = s2 @ sign(w3).T + b3; y3 = clip(bn3(h3), -1, 1)
  z  = y3 @ w4.T + b4;       out = log_softmax(z)

Sharding: pure data-parallel over the batch across 8 NeuronCores
(weights replicated, no collectives).

Numerics:
  - L1 splits x on the HOST into xa = fp16(x) plus a scaled fp8 residual
    rb = fp8e4((x - xa) * 2^9).  The residual's stationary operand is
    sign(w1) * 2^-9 (exactly representable in fp8e4), so rb @ (S1*2^-9)
    accumulates the residual at its true scale into the SAME fp32 PSUM
    as the fp16 stream -- no combine step.  Total L1 error <= 2^-15|x|,
    which measures 7.6e-3 rel err end-to-end on the actual inputs
    (tolerance 2e-2; the chaotic sign-flip cascade is the amplifier).
    The fp16 stream is 7 uniform 112-row passes (784 = 7*112); the
    residual is 3 DoubleRow passes of [112, 2] pairs (224 rows each)
    plus one plain-fp8 112-row pass -- fp8 DR does 2 rows/cycle, so the
    residual costs ~4 half-width passes instead of 7.
  - L2/L3: both operands are exactly +-1/0 in fp8e4 -> DoubleRow fp8
    matmuls produce bit-exact integer sums in fp32 PSUM.
  - BN + bias folding: bn(h + b) = A*h + C with A = g*rsqrt(v+eps),
    C = A*(b - m) + beta, applied per-partition by the Sign/Identity
    activations (fp32 internally).

Schedule notes (from NTFF trace analysis):
  - 112-row fp16 matmuls pace at the N-cycle rate (~216ns at N=512) even
    with a distinct LDWEIGHTS per matmul; 128-row ones pay +40ns, and
    512-wide DoubleRow pays +0.1ns/col, so L1 uses 112-row tiles and
    L2/L3 stay 256 columns wide.
  - startup: the critical path to the first MM is one x k-tile (112KB)
    plus one 512-column block of w1 (112KB); w1 is DMA'd in (k-tile,
    512-col) blocks interleaved with the group-0 x tiles.
  - a dummy-matmul burst on a zero tile warms the PE HAM clock gate
    (1.2 -> 2.4 GHz) while the startup DMAs are still in flight.
  - the log-softmax epilogue for group g-1 hides under group g; in the
    last group it is interleaved per batch-tile behind L4.
"""

import sys

if "/opt/trn_rl_repo" not in sys.path:
    sys.path.insert(0, "/opt/trn_rl_repo")

import numpy as np

D_IN, H1, H2, H3, NCLS = 784, 3072, 1536, 768, 10
B, NCORES = 16384, 8
BC = B // NCORES          # batch rows per core
W = 512                   # batch columns per group
NG = BC // W              # groups per core
KP = 112                  # L1 k-tile partition size (784 = 7 * 112)
K1T = D_IN // KP          # 7 fp16 passes
QR = 3                    # residual DoubleRow passes ([112, 2] pairs)
M1, M2, M3 = H1 // 128, H2 // 128, H3 // 128   # 24, 12, 6
NB1 = M1 // 4             # L1 column blocks of 512: 6
K2P, K3P = H1 // 256, H2 // 256                # DoubleRow k-pair iters: 12, 6
K4T = H3 // 128                                # 6
NCST = 2 * M1 + 2 * M2 + 2 * M3 + NCLS         # packed per-tile consts: 94
BN_EPS = 1e-5
RSH = 9                   # residual scale: rb = fp8(r * 2^RSH), w = +-2^-RSH
NWARM = 32

_cached = {}


def _build(bc):
    import concourse.bacc as bacc
    import concourse.mybir as mybir
    import concourse.tile as tile

    dt = mybir.dt
    AF = mybir.ActivationFunctionType
    PM = mybir.MatmulPerfMode
    ALU = mybir.AluOpType

    ng = bc // W
    nc = bacc.Bacc("TRN2", target_bir_lowering=False, debug=False,
                   num_devices=NCORES)

    # xa rows: per group, 7 k-tiles of 112 partitions
    xa = nc.declare_dram_parameter("xa", [ng * K1T * KP, W], dt.float16,
                                   isOutput=False)
    # residual rows: per group, 3 pair-tiles [112, 2, 512] then one plain
    # [112, 512]; 2D layout row=(g*4+q)*112+p, col=e*512+w (plain: e=0)
    xr = nc.declare_dram_parameter("xr", [ng * 4 * KP, 2 * W], dt.float8e4,
                                   isOutput=False)
    w1t = nc.declare_dram_parameter("w1t", [D_IN, H1], dt.float16,
                                    isOutput=False)
    # residual weights sign(w1).T * 2^-RSH: pair tiles row=q*112+p,
    # col=e*3072+n, then plain rows 672:784 at e=0
    w1r = nc.declare_dram_parameter("w1r", [4 * KP, 2 * H1], dt.float8e4,
                                    isOutput=False)
    w2t = nc.declare_dram_parameter("w2t", [H1, H2], dt.float8e4,
                                    isOutput=False)
    w3t = nc.declare_dram_parameter("w3t", [H2, H3], dt.float8e4,
                                    isOutput=False)
    w4t = nc.declare_dram_parameter("w4t", [H3, NCLS], dt.bfloat16,
                                    isOutput=False)
    cst = nc.declare_dram_parameter("cst", [128, NCST], dt.float32,
                                    isOutput=False)
    wrm = nc.declare_dram_parameter("wrm", [128, W], dt.float16,
                                    isOutput=False)
    out = nc.declare_dram_parameter("out", [bc, NCLS], dt.float32,
                                    isOutput=True)

    with tile.TileContext(nc) as tc, \
            tc.tile_pool(name="wts", bufs=1) as wp, \
            tc.tile_pool(name="xi0", bufs=1) as xp0, \
            tc.tile_pool(name="xin", bufs=2) as xp, \
            tc.tile_pool(name="act", bufs=2) as ap_, \
            tc.tile_pool(name="eps", bufs=2) as ep, \
            tc.tile_pool(name="ps", bufs=4, space="PSUM") as ps, \
            tc.tile_pool(name="ps4", bufs=2, space="PSUM") as ps4, \
            tc.tile_pool(name="psw", bufs=1, space="PSUM") as pw:

        # ---- HAM warm-up: burn the PE clock gate from 4/8 to 8/8 while
        # the startup DMAs stream.  Reads a 128KB zero tile (first DMA in
        # the ring), writes a dedicated PSUM bank that is never read.
        wrmb = wp.tile([128, W], dt.float16, tag="wrm")
        nc.sync.dma_start(wrmb[:], wrm[:])
        pwt = pw.tile([128, W], dt.float32, tag="pw")
        for _ in range(NWARM):
            nc.tensor.matmul(pwt[:], wrmb[:, 0:128], wrmb[:],
                             start=True, stop=True)

        # ---- group-0 x tiles and the first 512-col block of w1,
        # interleaved so MM (kt, mt<4) unblocks as soon as its own pair
        # of 112KB transfers lands.
        w1b = [[None] * NB1 for _ in range(K1T)]   # fp16 [kt][colblock]
        x0 = []
        for k in range(K1T):
            xj = xp0.tile([KP, W], dt.float16, tag=f"x0_{k}", name=f"x0_{k}")
            nc.sync.dma_start(xj[:], xa[k * KP:(k + 1) * KP, :])
            x0.append(xj)
            wkb = wp.tile([KP, W], dt.float16, tag=f"w1_{k}_0",
                          name=f"w1_{k}_0")
            nc.sync.dma_start(wkb[:], w1t[k * KP:(k + 1) * KP, 0:W])
            w1b[k][0] = wkb
        cstb = wp.tile([128, NCST], dt.float32, tag="cst")
        nc.sync.dma_start(cstb[:], cst[:])

        # group-0 residual moving tiles (needed ~6us after the first MM)
        xr0 = []
        for q in range(QR):
            t = xp0.tile([KP, 2, W], dt.float8e4, tag=f"xr0_{q}",
                         name=f"xr0_{q}")
            nc.sync.dma_start(
                t[:], xr[q * KP:(q + 1) * KP, :].rearrange(
                    "p (e w) -> p e w", e=2))
            xr0.append(t)
        t = xp0.tile([KP, W], dt.float8e4, tag="xr0_3", name="xr0_3")
        nc.sync.dma_start(t[:], xr[QR * KP:4 * KP, 0:W])
        xr0.append(t)

        # residual weights, first column block
        wrb = [[None] * NB1 for _ in range(QR + 1)]  # [q][colblock]
        for q in range(QR):
            t = wp.tile([KP, 2, W], dt.float8e4, tag=f"w1r_{q}_0",
                        name=f"w1r_{q}_0")
            for e in range(2):
                nc.sync.dma_start(t[:, e, :],
                                  w1r[q * KP:(q + 1) * KP,
                                      e * H1:e * H1 + W])
            wrb[q][0] = t
        t = wp.tile([KP, W], dt.float8e4, tag="w1r_3_0", name="w1r_3_0")
        nc.sync.dma_start(t[:], w1r[QR * KP:4 * KP, 0:W])
        wrb[QR][0] = t

        # const views into the packed per-output-tile scale/bias table
        def a1v(mt): return cstb[:, mt:mt + 1]
        def c1v(mt): return cstb[:, M1 + mt:M1 + mt + 1]
        def a2v(mt): return cstb[:, 2 * M1 + mt:2 * M1 + mt + 1]
        def c2v(mt): return cstb[:, 2 * M1 + M2 + mt:2 * M1 + M2 + mt + 1]
        def a3v(mt):
            o = 2 * M1 + 2 * M2
            return cstb[:, o + mt:o + mt + 1]
        def c3v(mt):
            o = 2 * M1 + 2 * M2 + M3
            return cstb[:, o + mt:o + mt + 1]
        b4v = cstb[:, NCST - NCLS:NCST]

        # ---- remaining w1/w1r column blocks, then w2/w3/w4; ring order
        # keeps every transfer ahead of its first consumer.
        for b in range(1, NB1):
            for k in range(K1T):
                wkb = wp.tile([KP, W], dt.float16, tag=f"w1_{k}_{b}",
                              name=f"w1_{k}_{b}")
                nc.sync.dma_start(wkb[:],
                                  w1t[k * KP:(k + 1) * KP, b * W:(b + 1) * W])
                w1b[k][b] = wkb
            for q in range(QR):
                t = wp.tile([KP, 2, W], dt.float8e4, tag=f"w1r_{q}_{b}",
                            name=f"w1r_{q}_{b}")
                for e in range(2):
                    nc.sync.dma_start(t[:, e, :],
                                      w1r[q * KP:(q + 1) * KP,
                                          e * H1 + b * W:e * H1 + (b + 1) * W])
                wrb[q][b] = t
            t = wp.tile([KP, W], dt.float8e4, tag=f"w1r_3_{b}",
                        name=f"w1r_3_{b}")
            nc.sync.dma_start(t[:], w1r[QR * KP:4 * KP, b * W:(b + 1) * W])
            wrb[QR][b] = t

        w2sb = wp.tile([128, 2 * K2P, H2], dt.float8e4, tag="w2")
        for kt in range(2 * K2P):
            nc.sync.dma_start(w2sb[:, kt, :],
                              w2t[kt * 128:(kt + 1) * 128, :])
        w3sb = wp.tile([128, 2 * K3P, H3], dt.float8e4, tag="w3")
        for kt in range(2 * K3P):
            nc.sync.dma_start(w3sb[:, kt, :],
                              w3t[kt * 128:(kt + 1) * 128, :])
        w4sb = wp.tile([128, K4T, NCLS], dt.bfloat16, tag="w4")
        nc.sync.dma_start(w4sb[:],
                          w4t.ap().rearrange("(kt p) n -> p kt n", p=128))

        zout = wp.tile([128, ng * 4, NCLS], dt.float32, tag="zout")
        ssum = wp.tile([128, ng * 4], dt.float32, tag="ssum")
        lsum = wp.tile([128, ng * 4], dt.float32, tag="lsum")

        def emit_epilogue(lo, hi):
            # log_softmax over the free dim; |z| is small so no max-shift
            for r in range(lo, hi):
                e = ep.tile([128, NCLS], dt.float32, tag="e")
                nc.scalar.activation(e[:], zout[:, r, :], AF.Exp,
                                     accum_out=ssum[:, r:r + 1])
            nc.scalar.activation(lsum[:, lo:hi], ssum[:, lo:hi], AF.Ln)
            for r in range(lo, hi):
                nc.vector.tensor_scalar(zout[:, r, :], zout[:, r, :],
                                        lsum[:, r:r + 1], None,
                                        op0=ALU.subtract)
            nc.sync.dma_start(
                out.ap()[lo * 128:hi * 128, :].rearrange(
                    "(g p) n -> p g n", p=128),
                zout[:, lo:hi, :])

        def l1_resid(pt, b, i, xrt):
            # residual accumulation into pt (start=False: the fp16 stream
            # already cleared the bank); per 256-col half, 3 DR pair
            # passes + 1 plain fp8 pass
            cs = slice(i * 128, (i + 1) * 128)
            for h in range(2):
                hs = slice(h * 256, (h + 1) * 256)
                for q in range(QR):
                    nc.tensor.matmul(pt[:, hs], wrb[q][b][:, :, cs],
                                     xrt[q][:, :, hs],
                                     start=False, stop=False,
                                     perf_mode=PM.DoubleRow)
                nc.tensor.matmul(pt[:, hs], wrb[QR][b][:, cs],
                                 xrt[QR][:, hs],
                                 start=False, stop=True)

        for g in range(ng):
            if g == 0:
                xtiles, xrt = x0, xr0
            else:
                xgt = xp.tile([KP, K1T, W], dt.float16, tag="xin")
                nc.sync.dma_start(
                    xgt[:], xa[g * K1T * KP:(g + 1) * K1T * KP, :].rearrange(
                        "(t p) w -> p t w", p=KP))
                xrg = xp.tile([KP, QR, 2, W], dt.float8e4, tag="xr")
                nc.sync.dma_start(
                    xrg[:], xr[g * 4 * KP:(g * 4 + QR) * KP, :].rearrange(
                        "(q p) (e w) -> p q e w", p=KP, e=2))
                xrl = xp.tile([KP, W], dt.float8e4, tag="xrl")
                nc.sync.dma_start(
                    xrl[:], xr[(g * 4 + QR) * KP:(g + 1) * 4 * KP, 0:W])
                xtiles = [xgt[:, t, :] for t in range(K1T)]
                xrt = [xrg[:, q, :, :] for q in range(QR)] + [xrl[:]]
                # epilogue for the previous group hides under this group's
                # L1 matmuls (issued after the x DMAs so the in-order DMA
                # ring never parks a not-yet-ready out transfer ahead of
                # a prefetch)
                emit_epilogue(4 * (g - 1), 4 * g)

            # ---- L1: [784 -> 3072], 7 fp16 passes + fp8 residual
            h1sb = ap_.tile([128, 2 * K2P, W], dt.float8e4, tag="h1")
            if g == 0:
                # k-outer over blocks of 4 PSUM banks: the PE consumes
                # each (x, w1) k-tile pair as its DMA lands instead of
                # idling until the whole stream arrives.
                for b in range(NB1):
                    pts = [ps.tile([128, W], dt.float32, tag="ps",
                                   name=f"pt{i}") for i in range(4)]
                    for k in range(K1T):
                        lhs = w1b[k][b]
                        for i in range(4):
                            nc.tensor.matmul(pts[i][:],
                                             lhs[:, i * 128:(i + 1) * 128],
                                             xtiles[k][:],
                                             start=(k == 0), stop=False)
                    for i in range(4):
                        l1_resid(pts[i], b, i, xrt)
                        mt = 4 * b + i
                        nc.scalar.activation(h1sb[:, mt, :], pts[i][:],
                                             AF.Sign, bias=c1v(mt),
                                             scale=a1v(mt))
            else:
                for mt in range(M1):
                    b, i = mt // 4, mt % 4
                    pt = ps.tile([128, W], dt.float32, tag="ps")
                    for k in range(K1T):
                        nc.tensor.matmul(pt[:],
                                         w1b[k][b][:, i * 128:(i + 1) * 128],
                                         xtiles[k][:],
                                         start=(k == 0), stop=False)
                    l1_resid(pt, b, i, xrt)
                    nc.scalar.activation(h1sb[:, mt, :], pt[:], AF.Sign,
                                         bias=c1v(mt), scale=a1v(mt))

            # ---- L2-L4 iterate over 256-column halves of the group (a
            # 512-wide DoubleRow matmul paces ~0.52 ns/col vs 0.425 at 256)
            for h in range(2):
                hs = slice(h * 256, (h + 1) * 256)
                # ---- L2: [3072 -> 1536], fp8 DoubleRow
                h2sb = ap_.tile([128, 2 * K3P, 256], dt.float8e4, tag="h2")
                for mt in range(M2):
                    pt = ps.tile([128, 256], dt.float32, tag="ps")
                    for kp in range(K2P):
                        nc.tensor.matmul(
                            pt[:],
                            w2sb[:, 2 * kp:2 * kp + 2,
                                 mt * 128:(mt + 1) * 128],
                            h1sb[:, 2 * kp:2 * kp + 2, hs],
                            start=(kp == 0), stop=(kp == K2P - 1),
                            perf_mode=PM.DoubleRow)
                    nc.scalar.activation(h2sb[:, mt, :], pt[:], AF.Sign,
                                         bias=c2v(mt), scale=a2v(mt))

                # ---- L3: [1536 -> 768], fp8 DoubleRow; scale/bias on the
                # Scalar engine (Identity), clip on DVE; bf16 output keeps
                # L4 single-pass (fp32 moving data double-pumps the PE)
                h3c = ap_.tile([128, K4T, 256], dt.bfloat16, tag="h3")
                for mt in range(M3):
                    pt = ps.tile([128, 256], dt.float32, tag="ps")
                    for kp in range(K3P):
                        nc.tensor.matmul(
                            pt[:],
                            w3sb[:, 2 * kp:2 * kp + 2,
                                 mt * 128:(mt + 1) * 128],
                            h2sb[:, 2 * kp:2 * kp + 2, :],
                            start=(kp == 0), stop=(kp == K3P - 1),
                            perf_mode=PM.DoubleRow)
                    nc.scalar.activation(h3c[:, mt, :], pt[:], AF.Identity,
                                         bias=c3v(mt), scale=a3v(mt))
                    nc.vector.tensor_scalar(h3c[:, mt, :], h3c[:, mt, :],
                                            1.0, -1.0, op0=ALU.min,
                                            op1=ALU.max)

                # ---- L4: logits z = y3 @ w4.T + b4, [batch-tile, 10]
                for bt in range(2):
                    r = 4 * g + 2 * h + bt
                    p4 = ps4.tile([128, NCLS], dt.float32, tag="p4")
                    for kt in range(K4T):
                        nc.tensor.matmul(p4[:],
                                         h3c[:, kt, bt * 128:(bt + 1) * 128],
                                         w4sb[:, kt, :],
                                         start=(kt == 0),
                                         stop=(kt == K4T - 1))
                    nc.vector.tensor_add(zout[:, r, :], p4[:], b4v)
                    if g == ng - 1:
                        # last group: per-tile epilogue rides behind the
                        # next batch-tile's L4 matmuls; only the last
                        # tile's short chain runs after the final MM
                        emit_epilogue(r, r + 1)

    nc.finalize()
    return nc


def _prep(x, w1, b1, w2, b2, w3, b3, w4, b4,
          g1, be1, m1, v1, g2, be2, m2, v2, g3, be3, m3, v3):
    """Host-side layout prep: transposes, binarized weight casts, BN folds,
    the fp16 split of x, and the scaled-fp8 residual repack."""
    import concourse.mybir as mybir
    f8 = mybir.dt.np(mybir.dt.float8e4)
    bf16 = mybir.dt.np(mybir.dt.bfloat16)

    def fold(g, be, m, v, b):
        a = (g / np.sqrt(v + np.float32(BN_EPS))).astype(np.float32)
        c = (a * (b - m) + be).astype(np.float32)
        return a, c

    a1, c1 = fold(g1, be1, m1, v1, b1)
    a2, c2 = fold(g2, be2, m2, v2, b2)
    a3, c3 = fold(g3, be3, m3, v3, b3)

    def cols(v, mtiles):
        return v.reshape(mtiles, 128).T

    cstm = np.zeros((128, NCST), np.float32)
    o = 0
    for v, m in ((a1, M1), (c1, M1), (a2, M2), (c2, M2), (a3, M3), (c3, M3)):
        cstm[:, o:o + m] = cols(v, m)
        o += m
    cstm[:, o:o + NCLS] = b4.astype(np.float32)[None, :]

    s1t = np.sign(w1).T.astype(np.float32)          # [784, 3072]
    s1r = (s1t * np.float32(2.0 ** -RSH)).astype(f8)
    # residual weight pack: pair tiles (q<3): row q*112+p, col e*3072+n
    # holds s1r[224q + p + 112e]; plain tile: row 336+p col n holds
    # s1r[672+p]
    w1rm = np.zeros((4 * KP, 2 * H1), f8)
    for q in range(QR):
        for e in range(2):
            w1rm[q * KP:(q + 1) * KP, e * H1:(e + 1) * H1] = \
                s1r[224 * q + 112 * e:224 * q + 112 * e + KP]
    w1rm[QR * KP:4 * KP, 0:H1] = s1r[672:784]

    pre = dict(
        w1t=np.ascontiguousarray(s1t.astype(np.float16)),
        w1r=w1rm,
        w2t=np.ascontiguousarray(np.sign(w2).T).astype(f8),
        w3t=np.ascontiguousarray(np.sign(w3).T).astype(f8),
        w4t=np.ascontiguousarray(w4.T).astype(bf16),
        cst=cstm,
        wrm=np.zeros((128, W), np.float16),
    )

    # fp16 part and scaled residual of x, transposed and repacked
    xa16 = x.astype(np.float16)
    r = x.astype(np.float32) - xa16.astype(np.float32)
    rb = (r * np.float32(2.0 ** RSH)).astype(f8)
    xat = xa16.T.reshape(D_IN, NCORES, NG, W)
    rbt = rb.T.reshape(D_IN, NCORES, NG, W)

    xap = np.empty((NCORES, NG, K1T, KP, W), np.float16)
    for t in range(K1T):
        xap[:, :, t] = xat[t * KP:(t + 1) * KP].transpose(1, 2, 0, 3)
    xrp = np.zeros((NCORES, NG, 4, KP, 2, W), f8)
    for q in range(QR):
        for e in range(2):
            xrp[:, :, q, :, e, :] = \
                rbt[224 * q + 112 * e:224 * q + 112 * e + KP].transpose(
                    1, 2, 0, 3)
    xrp[:, :, QR, :, 0, :] = rbt[672:784].transpose(1, 2, 0, 3)
    return pre, xap, xrp


def run(inputs, **spmd_kwargs):
    from concourse.bass_utils import run_bass_kernel_spmd

    if "nc" not in _cached:
        _cached["nc"] = _build(BC)
    nc = _cached["nc"]

    inputs = {k: np.asarray(v) for k, v in inputs.items()}
    pre, xap, xrp = _prep(**inputs)

    in_maps = []
    for core in range(NCORES):
        m = dict(pre)
        m["xa"] = np.ascontiguousarray(xap[core].reshape(NG * K1T * KP, W))
        m["xr"] = np.ascontiguousarray(xrp[core].reshape(NG * 4 * KP, 2 * W))
        in_maps.append(m)

    res = run_bass_kernel_spmd(nc, in_maps, list(range(NCORES)), **spmd_kwargs)
    outs = [res.results[i]["out"] for i in range(NCORES)]
    return res, np.concatenate(outs, axis=0).astype(np.float32)


def kernel(**inputs):
    return run(inputs)[1]


# revision 14
# speedup vs baseline: 1.3343x; 1.0031x over previous
"""Trainium2 Bass kernel for a binarized 4-layer MLP (eval mode).

Reference computation (per row of x [B=16384, 784]):
  h1 = x @ sign(w1).T + b1;  s1 = sign(bn1(h1))        (clip doesn't change sign)
  h2 = s1 @ sign(w2).T + b2; s2 = sign(bn2(h2))
  h3 = s2 @ sign(w3).T + b3; y3 = clip(bn3(h3), -1, 1)
  z  = y3 @ w4.T + b4;       out = log_softmax(z)

Sharding: pure data-parallel over the batch across 8 NeuronCores
(weights replicated, no collectives).

Numerics:
  - L1 splits x on the HOST into xa = fp16(x) plus a scaled fp8 residual
    rb = fp8e4((x - xa) * 2^9).  The residual's stationary operand is
    sign(w1) * 2^-9 (exactly representable in fp8e4), so rb @ (S1*2^-9)
    accumulates the residual at its true scale into the SAME fp32 PSUM
    as the fp16 stream -- no combine step.  Total L1 error <= 2^-15|x|,
    which measures 7.6e-3 rel err end-to-end on the actual inputs
    (tolerance 2e-2; the chaotic sign-flip cascade is the amplifier).
    The fp16 stream is 7 uniform 112-row passes (784 = 7*112); the
    residual is 3 DoubleRow passes of [112, 2] pairs (224 rows each)
    plus one plain-fp8 112-row pass -- fp8 DR does 2 rows/cycle, so the
    residual costs ~4 half-width passes instead of 7.
  - L2/L3: both operands are exactly +-1/0 in fp8e4 -> DoubleRow fp8
    matmuls produce bit-exact integer sums in fp32 PSUM.
  - BN + bias folding: bn(h + b) = A*h + C with A = g*rsqrt(v+eps),
    C = A*(b - m) + beta, applied per-partition by the Sign/Identity
    activations (fp32 internally).

Schedule notes (from NTFF trace analysis):
  - 112-row fp16 matmuls pace at the N-cycle rate (~216ns at N=512) even
    with a distinct LDWEIGHTS per matmul; 128-row ones pay +40ns, and
    512-wide DoubleRow pays +0.1ns/col, so L1 uses 112-row tiles and
    L2/L3 stay 256 columns wide.
  - startup: the critical path to the first MM is one x k-tile (112KB)
    plus one 512-column block of w1 (112KB); w1 is DMA'd in (k-tile,
    512-col) blocks interleaved with the group-0 x tiles.
  - a dummy-matmul burst on a zero tile warms the PE HAM clock gate
    (1.2 -> 2.4 GHz) while the startup DMAs are still in flight.
  - the log-softmax epilogue for group g-1 hides under group g; in the
    last group it is interleaved per batch-tile behind L4.
"""

import sys

if "/opt/trn_rl_repo" not in sys.path:
    sys.path.insert(0, "/opt/trn_rl_repo")

import numpy as np

D_IN, H1, H2, H3, NCLS = 784, 3072, 1536, 768, 10
B, NCORES = 16384, 8
BC = B // NCORES          # batch rows per core
W = 512                   # batch columns per group
NG = BC // W              # groups per core
KP = 112                  # L1 k-tile partition size (784 = 7 * 112)
K1T = D_IN // KP          # 7 fp16 passes
QR = 3                    # residual DoubleRow passes ([112, 2] pairs)
M1, M2, M3 = H1 // 128, H2 // 128, H3 // 128   # 24, 12, 6
NB1 = M1 // 4             # L1 column blocks of 512: 6
K2P, K3P = H1 // 256, H2 // 256                # DoubleRow k-pair iters: 12, 6
K4T = H3 // 128                                # 6
NCST = 2 * M1 + 2 * M2 + 2 * M3 + NCLS         # packed per-tile consts: 94
BN_EPS = 1e-5
RSH = 9                   # residual scale: rb = fp8(r * 2^RSH), w = +-2^-RSH
NWARM = 3

_cached = {}


def _build(bc):
    import concourse.bacc as bacc
    import concourse.mybir as mybir
    import concourse.tile as tile

    dt = mybir.dt
    AF = mybir.ActivationFunctionType
    PM = mybir.MatmulPerfMode
    ALU = mybir.AluOpType

    ng = bc // W
    nc = bacc.Bacc("TRN2", target_bir_lowering=False, debug=False,
                   num_devices=NCORES)

    # xa rows: per group, 7 k-tiles of 112 partitions
    xa = nc.declare_dram_parameter("xa", [ng * K1T * KP, W], dt.float16,
                                   isOutput=False)
    # residual rows: per group, 3 pair-tiles [112, 2, 512] then one plain
    # [112, 512]; 2D layout row=(g*4+q)*112+p, col=e*512+w (plain: e=0)
    xr = nc.declare_dram_parameter("xr", [ng * 4 * KP, 2 * W], dt.float8e4,
                                   isOutput=False)
    w1t = nc.declare_dram_parameter("w1t", [D_IN, H1], dt.float16,
                                    isOutput=False)
    # residual weights sign(w1).T * 2^-RSH: pair tiles row=q*112+p,
    # col=e*3072+n, then plain rows 672:784 at e=0
    w1r = nc.declare_dram_parameter("w1r", [4 * KP, 2 * H1], dt.float8e4,
                                    isOutput=False)
    w2t = nc.declare_dram_parameter("w2t", [H1, H2], dt.float8e4,
                                    isOutput=False)
    w3t = nc.declare_dram_parameter("w3t", [H2, H3], dt.float8e4,
                                    isOutput=False)
    w4t = nc.declare_dram_parameter("w4t", [H3, NCLS], dt.bfloat16,
                                    isOutput=False)
    cst = nc.declare_dram_parameter("cst", [128, NCST], dt.float32,
                                    isOutput=False)
    wrm = nc.declare_dram_parameter("wrm", [128, W], dt.float16,
                                    isOutput=False)
    out = nc.declare_dram_parameter("out", [bc, NCLS], dt.float32,
                                    isOutput=True)

    with tile.TileContext(nc) as tc, \
            tc.tile_pool(name="wts", bufs=1) as wp, \
            tc.tile_pool(name="xi0", bufs=1) as xp0, \
            tc.tile_pool(name="xin", bufs=2) as xp, \
            tc.tile_pool(name="act", bufs=2) as ap_, \
            tc.tile_pool(name="eps", bufs=2) as ep, \
            tc.tile_pool(name="ps", bufs=4, space="PSUM") as ps, \
            tc.tile_pool(name="ps4", bufs=2, space="PSUM") as ps4, \
            tc.tile_pool(name="psw", bufs=1, space="PSUM") as pw:

        # ---- HAM warm-up: burn the PE clock gate from 4/8 to 8/8 while
        # the startup DMAs stream.  Reads a 128KB zero tile (first DMA in
        # the ring), writes a dedicated PSUM bank that is never read.
        wrmb = wp.tile([128, W], dt.float16, tag="wrm")
        nc.sync.dma_start(wrmb[:], wrm[:])
        pwt = pw.tile([128, W], dt.float32, tag="pw")
        for wi in range(NWARM):
            nc.tensor.matmul(pwt[:], wrmb[:, 0:128], wrmb[:],
                             start=(wi == 0), stop=(wi == NWARM - 1))

        # ---- group-0 x tiles and the first 512-col block of w1,
        # interleaved so MM (kt, mt<4) unblocks as soon as its own pair
        # of 112KB transfers lands.
        w1b = [[None] * NB1 for _ in range(K1T)]   # fp16 [kt][colblock]
        x0 = []
        for k in range(K1T):
            xj = xp0.tile([KP, W], dt.float16, tag=f"x0_{k}", name=f"x0_{k}")
            nc.sync.dma_start(xj[:], xa[k * KP:(k + 1) * KP, :])
            x0.append(xj)
            wkb = wp.tile([KP, W], dt.float16, tag=f"w1_{k}_0",
                          name=f"w1_{k}_0")
            nc.sync.dma_start(wkb[:], w1t[k * KP:(k + 1) * KP, 0:W])
            w1b[k][0] = wkb
        cstb = wp.tile([128, NCST], dt.float32, tag="cst")
        nc.sync.dma_start(cstb[:], cst[:])

        # group-0 residual moving tiles (needed ~6us after the first MM)
        xr0 = []
        for q in range(QR):
            t = xp0.tile([KP, 2, W], dt.float8e4, tag=f"xr0_{q}",
                         name=f"xr0_{q}")
            nc.sync.dma_start(
                t[:], xr[q * KP:(q + 1) * KP, :].rearrange(
                    "p (e w) -> p e w", e=2))
            xr0.append(t)
        t = xp0.tile([KP, W], dt.float8e4, tag="xr0_3", name="xr0_3")
        nc.sync.dma_start(t[:], xr[QR * KP:4 * KP, 0:W])
        xr0.append(t)

        # residual weights, first column block
        wrb = [[None] * NB1 for _ in range(QR + 1)]  # [q][colblock]
        for q in range(QR):
            t = wp.tile([KP, 2, W], dt.float8e4, tag=f"w1r_{q}_0",
                        name=f"w1r_{q}_0")
            for e in range(2):
                nc.sync.dma_start(t[:, e, :],
                                  w1r[q * KP:(q + 1) * KP,
                                      e * H1:e * H1 + W])
            wrb[q][0] = t
        t = wp.tile([KP, W], dt.float8e4, tag="w1r_3_0", name="w1r_3_0")
        nc.sync.dma_start(t[:], w1r[QR * KP:4 * KP, 0:W])
        wrb[QR][0] = t

        # const views into the packed per-output-tile scale/bias table
        def a1v(mt): return cstb[:, mt:mt + 1]
        def c1v(mt): return cstb[:, M1 + mt:M1 + mt + 1]
        def a2v(mt): return cstb[:, 2 * M1 + mt:2 * M1 + mt + 1]
        def c2v(mt): return cstb[:, 2 * M1 + M2 + mt:2 * M1 + M2 + mt + 1]
        def a3v(mt):
            o = 2 * M1 + 2 * M2
            return cstb[:, o + mt:o + mt + 1]
        def c3v(mt):
            o = 2 * M1 + 2 * M2 + M3
            return cstb[:, o + mt:o + mt + 1]
        b4v = cstb[:, NCST - NCLS:NCST]

        # ---- remaining w1/w1r column blocks, then w2/w3/w4; ring order
        # keeps every transfer ahead of its first consumer.
        for b in range(1, NB1):
            for k in range(K1T):
                wkb = wp.tile([KP, W], dt.float16, tag=f"w1_{k}_{b}",
                              name=f"w1_{k}_{b}")
                nc.sync.dma_start(wkb[:],
                                  w1t[k * KP:(k + 1) * KP, b * W:(b + 1) * W])
                w1b[k][b] = wkb
            for q in range(QR):
                t = wp.tile([KP, 2, W], dt.float8e4, tag=f"w1r_{q}_{b}",
                            name=f"w1r_{q}_{b}")
                for e in range(2):
                    nc.sync.dma_start(t[:, e, :],
                                      w1r[q * KP:(q + 1) * KP,
                                          e * H1 + b * W:e * H1 + (b + 1) * W])
                wrb[q][b] = t
            t = wp.tile([KP, W], dt.float8e4, tag=f"w1r_3_{b}",
                        name=f"w1r_3_{b}")
            nc.sync.dma_start(t[:], w1r[QR * KP:4 * KP, b * W:(b + 1) * W])
            wrb[QR][b] = t

        w2sb = wp.tile([128, 2 * K2P, H2], dt.float8e4, tag="w2")
        for kt in range(2 * K2P):
            nc.sync.dma_start(w2sb[:, kt, :],
                              w2t[kt * 128:(kt + 1) * 128, :])
        w3sb = wp.tile([128, 2 * K3P, H3], dt.float8e4, tag="w3")
        for kt in range(2 * K3P):
            nc.sync.dma_start(w3sb[:, kt, :],
                              w3t[kt * 128:(kt + 1) * 128, :])
        w4sb = wp.tile([128, K4T, NCLS], dt.bfloat16, tag="w4")
        nc.sync.dma_start(w4sb[:],
                          w4t.ap().rearrange("(kt p) n -> p kt n", p=128))

        zout = wp.tile([128, ng * 4, NCLS], dt.float32, tag="zout")
        ssum = wp.tile([128, ng * 4], dt.float32, tag="ssum")
        lsum = wp.tile([128, ng * 4], dt.float32, tag="lsum")

        def emit_epilogue(lo, hi):
            # log_softmax over the free dim; |z| is small so no max-shift
            for r in range(lo, hi):
                e = ep.tile([128, NCLS], dt.float32, tag="e")
                nc.scalar.activation(e[:], zout[:, r, :], AF.Exp,
                                     accum_out=ssum[:, r:r + 1])
            nc.scalar.activation(lsum[:, lo:hi], ssum[:, lo:hi], AF.Ln)
            for r in range(lo, hi):
                nc.vector.tensor_scalar(zout[:, r, :], zout[:, r, :],
                                        lsum[:, r:r + 1], None,
                                        op0=ALU.subtract)
            nc.sync.dma_start(
                out.ap()[lo * 128:hi * 128, :].rearrange(
                    "(g p) n -> p g n", p=128),
                zout[:, lo:hi, :])

        def l1_resid(pt, b, i, xrt):
            # residual accumulation into pt (start=False: the fp16 stream
            # already cleared the bank); per 256-col half, 3 DR pair
            # passes + 1 plain fp8 pass
            cs = slice(i * 128, (i + 1) * 128)
            for h in range(2):
                hs = slice(h * 256, (h + 1) * 256)
                for q in range(QR):
                    nc.tensor.matmul(pt[:, hs], wrb[q][b][:, :, cs],
                                     xrt[q][:, :, hs],
                                     start=False, stop=False,
                                     perf_mode=PM.DoubleRow)
                nc.tensor.matmul(pt[:, hs], wrb[QR][b][:, cs],
                                 xrt[QR][:, hs],
                                 start=False, stop=True)

        for g in range(ng):
            if g == 0:
                xtiles, xrt = x0, xr0
            else:
                xgt = xp.tile([KP, K1T, W], dt.float16, tag="xin")
                nc.sync.dma_start(
                    xgt[:], xa[g * K1T * KP:(g + 1) * K1T * KP, :].rearrange(
                        "(t p) w -> p t w", p=KP))
                xrg = xp.tile([KP, QR, 2, W], dt.float8e4, tag="xr")
                nc.sync.dma_start(
                    xrg[:], xr[g * 4 * KP:(g * 4 + QR) * KP, :].rearrange(
                        "(q p) (e w) -> p q e w", p=KP, e=2))
                xrl = xp.tile([KP, W], dt.float8e4, tag="xrl")
                nc.sync.dma_start(
                    xrl[:], xr[(g * 4 + QR) * KP:(g + 1) * 4 * KP, 0:W])
                xtiles = [xgt[:, t, :] for t in range(K1T)]
                xrt = [xrg[:, q, :, :] for q in range(QR)] + [xrl[:]]
                # epilogue for the previous group hides under this group's
                # L1 matmuls (issued after the x DMAs so the in-order DMA
                # ring never parks a not-yet-ready out transfer ahead of
                # a prefetch)
                emit_epilogue(4 * (g - 1), 4 * g)

            # ---- L1: [784 -> 3072], 7 fp16 passes + fp8 residual
            # h1/h2/h3 are split into two tiles so the next layer's first
            # matmul depends only on the first half's activations
            # (tile-granular deps would stall the PE at every phase
            # boundary until the last Sign lands)
            h1t = [ap_.tile([128, K2P, W], dt.float8e4, tag="h1a",
                            name="h1a"),
                   ap_.tile([128, K2P, W], dt.float8e4, tag="h1b",
                            name="h1b")]
            if g == 0:
                # k-outer over blocks of 4 PSUM banks: the PE consumes
                # each (x, w1) k-tile pair as its DMA lands instead of
                # idling until the whole stream arrives.
                for b in range(NB1):
                    pts = [ps.tile([128, W], dt.float32, tag="ps",
                                   name=f"pt{i}") for i in range(4)]
                    for k in range(K1T):
                        lhs = w1b[k][b]
                        for i in range(4):
                            nc.tensor.matmul(pts[i][:],
                                             lhs[:, i * 128:(i + 1) * 128],
                                             xtiles[k][:],
                                             start=(k == 0), stop=False)
                    for i in range(4):
                        l1_resid(pts[i], b, i, xrt)
                        mt = 4 * b + i
                        nc.scalar.activation(
                            h1t[mt // 12][:, mt % 12, :], pts[i][:],
                            AF.Sign, bias=c1v(mt), scale=a1v(mt))
            else:
                for mt in range(M1):
                    b, i = mt // 4, mt % 4
                    pt = ps.tile([128, W], dt.float32, tag="ps")
                    for k in range(K1T):
                        nc.tensor.matmul(pt[:],
                                         w1b[k][b][:, i * 128:(i + 1) * 128],
                                         xtiles[k][:],
                                         start=(k == 0), stop=False)
                    l1_resid(pt, b, i, xrt)
                    nc.scalar.activation(h1t[mt // 12][:, mt % 12, :],
                                         pt[:], AF.Sign,
                                         bias=c1v(mt), scale=a1v(mt))

            # ---- L2-L4 iterate over 256-column halves of the group (a
            # 512-wide DoubleRow matmul paces ~0.52 ns/col vs 0.425 at 256)
            for h in range(2):
                hs = slice(h * 256, (h + 1) * 256)
                # ---- L2: [3072 -> 1536], fp8 DoubleRow
                h2t = [ap_.tile([128, K3P, 256], dt.float8e4, tag="h2a",
                                name="h2a"),
                       ap_.tile([128, K3P, 256], dt.float8e4, tag="h2b",
                                name="h2b")]
                for mt in range(M2):
                    pt = ps.tile([128, 256], dt.float32, tag="ps")
                    for kp in range(K2P):
                        nc.tensor.matmul(
                            pt[:],
                            w2sb[:, 2 * kp:2 * kp + 2,
                                 mt * 128:(mt + 1) * 128],
                            h1t[kp // 6][:, 2 * (kp % 6):2 * (kp % 6) + 2,
                                          hs],
                            start=(kp == 0), stop=(kp == K2P - 1),
                            perf_mode=PM.DoubleRow)
                    nc.scalar.activation(h2t[mt // 6][:, mt % 6, :], pt[:],
                                         AF.Sign, bias=c2v(mt),
                                         scale=a2v(mt))

                # ---- L3: [1536 -> 768], fp8 DoubleRow; scale/bias on the
                # Scalar engine (Identity), clip on DVE; bf16 output keeps
                # L4 single-pass (fp32 moving data double-pumps the PE)
                h3t = [ap_.tile([128, K4T // 2, 256], dt.bfloat16,
                                tag="h3a", name="h3a"),
                       ap_.tile([128, K4T // 2, 256], dt.bfloat16,
                                tag="h3b", name="h3b")]
                for mt in range(M3):
                    pt = ps.tile([128, 256], dt.float32, tag="ps")
                    for kp in range(K3P):
                        nc.tensor.matmul(
                            pt[:],
                            w3sb[:, 2 * kp:2 * kp + 2,
                                 mt * 128:(mt + 1) * 128],
                            h2t[kp // 3][:, 2 * (kp % 3):2 * (kp % 3) + 2,
                                          :],
                            start=(kp == 0), stop=(kp == K3P - 1),
                            perf_mode=PM.DoubleRow)
                    h3v = h3t[mt // 3][:, mt % 3, :]
                    nc.scalar.activation(h3v, pt[:], AF.Identity,
                                         bias=c3v(mt), scale=a3v(mt))
                    nc.vector.tensor_scalar(h3v, h3v, 1.0, -1.0,
                                            op0=ALU.min, op1=ALU.max)

                # ---- L4: logits z = y3 @ w4.T + b4, [batch-tile, 10]
                for bt in range(2):
                    r = 4 * g + 2 * h + bt
                    p4 = ps4.tile([128, NCLS], dt.float32, tag="p4")
                    for kt in range(K4T):
                        nc.tensor.matmul(p4[:],
                                         h3t[kt // 3][:, kt % 3,
                                             bt * 128:(bt + 1) * 128],
                                         w4sb[:, kt, :],
                                         start=(kt == 0),
                                         stop=(kt == K4T - 1))
                    nc.vector.tensor_add(zout[:, r, :], p4[:], b4v)
                    if g == ng - 1:
                        # last group: per-tile epilogue rides behind the
                        # next batch-tile's L4 matmuls; only the last
                        # tile's short chain runs after the final MM
                        emit_epilogue(r, r + 1)

    nc.finalize()
    return nc


def _prep(x, w1, b1, w2, b2, w3, b3, w4, b4,
          g1, be1, m1, v1, g2, be2, m2, v2, g3, be3, m3, v3):
    """Host-side layout prep: transposes, binarized weight casts, BN folds,
    the fp16 split of x, and the scaled-fp8 residual repack."""
    import concourse.mybir as mybir
    f8 = mybir.dt.np(mybir.dt.float8e4)
    bf16 = mybir.dt.np(mybir.dt.bfloat16)

    def fold(g, be, m, v, b):
        a = (g / np.sqrt(v + np.float32(BN_EPS))).astype(np.float32)
        c = (a * (b - m) + be).astype(np.float32)
        return a, c

    a1, c1 = fold(g1, be1, m1, v1, b1)
    a2, c2 = fold(g2, be2, m2, v2, b2)
    a3, c3 = fold(g3, be3, m3, v3, b3)

    def cols(v, mtiles):
        return v.reshape(mtiles, 128).T

    cstm = np.zeros((128, NCST), np.float32)
    o = 0
    for v, m in ((a1, M1), (c1, M1), (a2, M2), (c2, M2), (a3, M3), (c3, M3)):
        cstm[:, o:o + m] = cols(v, m)
        o += m
    cstm[:, o:o + NCLS] = b4.astype(np.float32)[None, :]

    s1t = np.sign(w1).T.astype(np.float32)          # [784, 3072]
    s1r = (s1t * np.float32(2.0 ** -RSH)).astype(f8)
    # residual weight pack: pair tiles (q<3): row q*112+p, col e*3072+n
    # holds s1r[224q + p + 112e]; plain tile: row 336+p col n holds
    # s1r[672+p]
    w1rm = np.zeros((4 * KP, 2 * H1), f8)
    for q in range(QR):
        for e in range(2):
            w1rm[q * KP:(q + 1) * KP, e * H1:(e + 1) * H1] = \
                s1r[224 * q + 112 * e:224 * q + 112 * e + KP]
    w1rm[QR * KP:4 * KP, 0:H1] = s1r[672:784]

    pre = dict(
        w1t=np.ascontiguousarray(s1t.astype(np.float16)),
        w1r=w1rm,
        w2t=np.ascontiguousarray(np.sign(w2).T).astype(f8),
        w3t=np.ascontiguousarray(np.sign(w3).T).astype(f8),
        w4t=np.ascontiguousarray(w4.T).astype(bf16),
        cst=cstm,
        wrm=np.zeros((128, W), np.float16),
    )

    # fp16 part and scaled residual of x, transposed and repacked
    xa16 = x.astype(np.float16)
    r = x.astype(np.float32) - xa16.astype(np.float32)
    rb = (r * np.float32(2.0 ** RSH)).astype(f8)
    xat = xa16.T.reshape(D_IN, NCORES, NG, W)
    rbt = rb.T.reshape(D_IN, NCORES, NG, W)

    xap = np.empty((NCORES, NG, K1T, KP, W), np.float16)
    for t in range(K1T):
        xap[:, :, t] = xat[t * KP:(t + 1) * KP].transpose(1, 2, 0, 3)
    xrp = np.zeros((NCORES, NG, 4, KP, 2, W), f8)
    for q in range(QR):
        for e in range(2):
            xrp[:, :, q, :, e, :] = \
                rbt[224 * q + 112 * e:224 * q + 112 * e + KP].transpose(
                    1, 2, 0, 3)
    xrp[:, :, QR, :, 0, :] = rbt[672:784].transpose(1, 2, 0, 3)
    return pre, xap, xrp


def run(inputs, **spmd_kwargs):
    from concourse.bass_utils import run_bass_kernel_spmd

    if "nc" not in _cached:
        _cached["nc"] = _build(BC)
    nc = _cached["nc"]

    inputs = {k: np.asarray(v) for k, v in inputs.items()}
    pre, xap, xrp = _prep(**inputs)

    in_maps = []
    for core in range(NCORES):
        m = dict(pre)
        m["xa"] = np.ascontiguousarray(xap[core].reshape(NG * K1T * KP, W))
        m["xr"] = np.ascontiguousarray(xrp[core].reshape(NG * 4 * KP, 2 * W))
        in_maps.append(m)

    res = run_bass_kernel_spmd(nc, in_maps, list(range(NCORES)), **spmd_kwargs)
    outs = [res.results[i]["out"] for i in range(NCORES)]
    return res, np.concatenate(outs, axis=0).astype(np.float32)


def kernel(**inputs):
    return run(inputs)[1]


# revision 17
# speedup vs baseline: 1.3620x; 1.0208x over previous
"""Trainium2 Bass kernel for a binarized 4-layer MLP (eval mode).

Reference computation (per row of x [B=16384, 784]):
  h1 = x @ sign(w1).T + b1;  s1 = sign(bn1(h1))        (clip doesn't change sign)
  h2 = s1 @ sign(w2).T + b2; s2 = sign(bn2(h2))
  h3 = s2 @ sign(w3).T + b3; y3 = clip(bn3(h3), -1, 1)
  z  = y3 @ w4.T + b4;       out = log_softmax(z)

Sharding: pure data-parallel over the batch across 8 NeuronCores
(weights replicated, no collectives).

Numerics:
  - L1 splits x on the HOST into xa = fp16(x) plus a scaled fp8 residual
    rb = fp8e4((x - xa) * 2^9).  The residual's stationary operand is
    sign(w1) * 2^-9 (exactly representable in fp8e4), so rb @ (S1*2^-9)
    accumulates the residual at its true scale into the SAME fp32 PSUM
    as the fp16 stream -- no combine step.  Total L1 error <= 2^-15|x|,
    which measures 7.6e-3 rel err end-to-end on the actual inputs
    (tolerance 2e-2; the chaotic sign-flip cascade is the amplifier).
    The fp16 stream is 7 uniform 112-row passes (784 = 7*112); the
    residual is 3 DoubleRow passes of [112, 2] row pairs (224 rows each)
    plus one plain-fp8 112-row pass -- fp8 DR does 2 rows/cycle, so the
    residual costs ~4 half-width passes instead of 7.
  - L2/L3: both operands are exactly +-1/0 in fp8e4 -> DoubleRow fp8
    matmuls produce bit-exact integer sums in fp32 PSUM.
  - BN + bias folding: bn(h + b) = A*h + C with A = g*rsqrt(v+eps),
    C = A*(b - m) + beta, applied per-partition by the Sign/Identity
    activations (fp32 internally).

Schedule notes (from NTFF trace analysis):
  - 112-row fp16 matmuls pace at the N-cycle rate (~216ns at N=512);
    128-row ones pay +40ns and 512-wide DoubleRow pays +0.1ns/col, so
    L1 uses 112-row tiles and L2/L3 stay 256 columns wide.
  - every dma_start costs ~620ns of serialized Sync-engine time to
    trigger regardless of size, so inputs/weights load as FEW large
    transfers covering all four batch groups at once; only the
    group-0 block-0 critical path keeps small fine-grained transfers.
  - a dummy-matmul burst on a zero tile warms the PE HAM clock gate
    (1.2 -> 2.4 GHz) while the startup DMAs are still in flight.
  - h1/h2/h3 activations are split into two tiles each so the next
    layer's first matmul depends only on the first half (tile-granular
    deps would stall the PE at every phase boundary).
  - the log-softmax epilogue for group g-1 hides under group g; in the
    last group it is interleaved per batch-tile behind L4.
"""

import sys

if "/opt/trn_rl_repo" not in sys.path:
    sys.path.insert(0, "/opt/trn_rl_repo")

import numpy as np

D_IN, H1, H2, H3, NCLS = 784, 3072, 1536, 768, 10
B, NCORES = 16384, 8
BC = B // NCORES          # batch rows per core
W = 512                   # batch columns per group
NG = BC // W              # groups per core
KP = 112                  # L1 k-tile partition size (784 = 7 * 112)
K1T = D_IN // KP          # 7 fp16 passes
QR = 3                    # residual DoubleRow passes ([112, 2] row pairs)
M1, M2, M3 = H1 // 128, H2 // 128, H3 // 128   # 24, 12, 6
K2P, K3P = H1 // 256, H2 // 256                # DoubleRow k-pair iters: 12, 6
K4T = H3 // 128                                # 6
NCST = 2 * M1 + 2 * M2 + 2 * M3 + NCLS         # packed per-tile consts: 94
BN_EPS = 1e-5
RSH = 9                   # residual scale: rb = fp8(r * 2^RSH), w = +-2^-RSH
NWARM = 3

_cached = {}


def _build(bc):
    import concourse.bacc as bacc
    import concourse.mybir as mybir
    import concourse.tile as tile

    dt = mybir.dt
    AF = mybir.ActivationFunctionType
    PM = mybir.MatmulPerfMode
    ALU = mybir.AluOpType

    ng = bc // W
    nc = bacc.Bacc("TRN2", target_bir_lowering=False, debug=False,
                   num_devices=NCORES)

    # xa = fp16(x).T per core [784, bc]; row k = feature k
    xa = nc.declare_dram_parameter("xa", [D_IN, bc], dt.float16,
                                   isOutput=False)
    # xr = fp8((x - xa) * 2^RSH).T per core [784, bc]; rows 224q..224(q+1)
    # form DoubleRow pair tile q ((e p) order), rows 672:784 the plain pass
    xr = nc.declare_dram_parameter("xr", [D_IN, bc], dt.float8e4,
                                   isOutput=False)
    w1t = nc.declare_dram_parameter("w1t", [D_IN, H1], dt.float16,
                                    isOutput=False)
    # w1r = sign(w1).T * 2^-RSH in fp8, same row scheme as xr
    w1r = nc.declare_dram_parameter("w1r", [D_IN, H1], dt.float8e4,
                                    isOutput=False)
    w2t = nc.declare_dram_parameter("w2t", [H1, H2], dt.float8e4,
                                    isOutput=False)
    w3t = nc.declare_dram_parameter("w3t", [H2, H3], dt.float8e4,
                                    isOutput=False)
    w4t = nc.declare_dram_parameter("w4t", [H3, NCLS], dt.bfloat16,
                                    isOutput=False)
    cst = nc.declare_dram_parameter("cst", [128, NCST], dt.float32,
                                    isOutput=False)
    wrm = nc.declare_dram_parameter("wrm", [128, W], dt.float16,
                                    isOutput=False)
    out = nc.declare_dram_parameter("out", [bc, NCLS], dt.float32,
                                    isOutput=True)

    with tile.TileContext(nc) as tc, \
            tc.tile_pool(name="wts", bufs=1) as wp, \
            tc.tile_pool(name="act", bufs=1) as ap_, \
            tc.tile_pool(name="eps", bufs=2) as ep, \
            tc.tile_pool(name="ps", bufs=4, space="PSUM") as ps, \
            tc.tile_pool(name="ps4", bufs=2, space="PSUM") as ps4, \
            tc.tile_pool(name="psw", bufs=1, space="PSUM") as pw:

        # ---- HAM warm-up: a short accumulate chain on a zero tile keeps
        # the PE busy while the startup DMAs stream so the clock gate
        # lifts (1.2 -> 2.4 GHz) sooner.
        wrmb = wp.tile([128, W], dt.float16, tag="wrm")
        nc.sync.dma_start(wrmb[:], wrm[:])
        pwt = pw.tile([128, W], dt.float32, tag="pw")
        for wi in range(NWARM):
            nc.tensor.matmul(pwt[:], wrmb[:, 0:128], wrmb[:],
                             start=(wi == 0), stop=(wi == NWARM - 1))

        # ---- startup-critical transfers first.  Trigger order is the
        # schedule: group-0 x and the first 512-col block of w1/w1r go
        # first (fine-grained), then one (w1, w1r) transfer per 512-col
        # block paced to the group-0 chain consumption, then the bulk.
        x00 = wp.tile([KP, W], dt.float16, tag="x00")
        nc.sync.dma_start(x00[:], xa[0:KP, 0:W])
        w1k0a = wp.tile([KP, W], dt.float16, tag="w1k0a")
        nc.sync.dma_start(w1k0a[:], w1t[0:KP, 0:W])
        x0r = wp.tile([KP, K1T - 1, W], dt.float16, tag="x0r")
        nc.sync.dma_start(x0r[:], xa[KP:D_IN, 0:W].rearrange(
            "(k p) w -> p k w", p=KP))
        w1ka = wp.tile([KP, K1T - 1, W], dt.float16, tag="w1ka")
        nc.sync.dma_start(w1ka[:], w1t[KP:D_IN, 0:W].rearrange(
            "(k p) w -> p k w", p=KP))
        cstb = wp.tile([128, NCST], dt.float32, tag="cst")
        nc.sync.dma_start(cstb[:], cst[:])
        # group-0 residual moving data: rows (2q+e)*112+p are the pair
        # tiles, rows 672:784 the plain pass -- natural feature order
        xr0 = wp.tile([KP, K1T, W], dt.float8e4, tag="xr0")
        nc.sync.dma_start(xr0[:], xr[:, 0:W].rearrange(
            "(j p) w -> p j w", p=KP))
        # w1 and residual weights per 512-col block (block 0 of w1 is the
        # two tiles above)
        wrcb = []
        t = wp.tile([KP, K1T, W], dt.float8e4, tag="wr0", name="wr0")
        nc.sync.dma_start(t[:], w1r[:, 0:W].rearrange(
            "(j p) n -> p j n", p=KP))
        wrcb.append(t)
        w1cb = [None]
        for b in range(1, M1 // 4):
            t = wp.tile([KP, K1T, W], dt.float16, tag=f"w1c{b}",
                        name=f"w1c{b}")
            nc.sync.dma_start(t[:], w1t[:, b * W:(b + 1) * W].rearrange(
                "(k p) n -> p k n", p=KP))
            w1cb.append(t)
            t = wp.tile([KP, K1T, W], dt.float8e4, tag=f"wr{b}",
                        name=f"wr{b}")
            nc.sync.dma_start(t[:], w1r[:, b * W:(b + 1) * W].rearrange(
                "(j p) n -> p j n", p=KP))
            wrcb.append(t)

        def w1v(k, mt):
            b, i = mt // 4, mt % 4
            cs = slice(i * 128, (i + 1) * 128)
            if b == 0:
                return w1k0a[:, cs] if k == 0 else w1ka[:, k - 1, cs]
            return w1cb[b][:, k, cs]

        # fp16 x and residual moving data for groups 1..3, one transfer
        xab = wp.tile([KP, K1T, (ng - 1) * W], dt.float16, tag="xab")
        nc.sync.dma_start(xab[:], xa[:, W:bc].rearrange(
            "(k p) w -> p k w", p=KP))
        xr123 = wp.tile([KP, K1T, (ng - 1) * W], dt.float8e4, tag="xr123")
        nc.sync.dma_start(xr123[:], xr[:, W:bc].rearrange(
            "(j p) w -> p j w", p=KP))

        w2sb = wp.tile([128, 2 * K2P, H2], dt.float8e4, tag="w2")
        nc.sync.dma_start(w2sb[:], w2t.ap().rearrange(
            "(t p) n -> p t n", p=128))
        w3sb = wp.tile([128, 2 * K3P, H3], dt.float8e4, tag="w3")
        nc.sync.dma_start(w3sb[:], w3t.ap().rearrange(
            "(t p) n -> p t n", p=128))
        w4sb = wp.tile([128, K4T, NCLS], dt.bfloat16, tag="w4")
        nc.sync.dma_start(w4sb[:],
                          w4t.ap().rearrange("(kt p) n -> p kt n", p=128))

        # const views into the packed per-output-tile scale/bias table
        def a1v(mt): return cstb[:, mt:mt + 1]
        def c1v(mt): return cstb[:, M1 + mt:M1 + mt + 1]
        def a2v(mt): return cstb[:, 2 * M1 + mt:2 * M1 + mt + 1]
        def c2v(mt): return cstb[:, 2 * M1 + M2 + mt:2 * M1 + M2 + mt + 1]
        def a3v(mt):
            o = 2 * M1 + 2 * M2
            return cstb[:, o + mt:o + mt + 1]
        def c3v(mt):
            o = 2 * M1 + 2 * M2 + M3
            return cstb[:, o + mt:o + mt + 1]
        b4v = cstb[:, NCST - NCLS:NCST]

        zout = wp.tile([128, ng * 4, NCLS], dt.float32, tag="zout")
        ssum = wp.tile([128, ng * 4], dt.float32, tag="ssum")
        lsum = wp.tile([128, ng * 4], dt.float32, tag="lsum")

        def emit_epilogue(lo, hi):
            # log_softmax over the free dim; |z| is small so no max-shift
            for r in range(lo, hi):
                e = ep.tile([128, NCLS], dt.float32, tag="e")
                nc.scalar.activation(e[:], zout[:, r, :], AF.Exp,
                                     accum_out=ssum[:, r:r + 1])
            nc.scalar.activation(lsum[:, lo:hi], ssum[:, lo:hi], AF.Ln)
            for r in range(lo, hi):
                nc.vector.tensor_scalar(zout[:, r, :], zout[:, r, :],
                                        lsum[:, r:r + 1], None,
                                        op0=ALU.subtract)
            nc.sync.dma_start(
                out.ap()[lo * 128:hi * 128, :].rearrange(
                    "(g p) n -> p g n", p=128),
                zout[:, lo:hi, :])

        def l1_resid(pt, mt, g):
            # residual accumulation into pt (start=False: the fp16 stream
            # already cleared the bank); per 256-col half, 3 DoubleRow
            # pair passes + 1 plain fp8 pass
            b, i = mt // 4, mt % 4
            cs = slice(i * 128, (i + 1) * 128)
            xv = xr0 if g == 0 else xr123
            off = 0 if g == 0 else (g - 1) * W
            for h in range(2):
                hs = slice(off + h * 256, off + (h + 1) * 256)
                for q in range(QR):
                    nc.tensor.matmul(pt[:, h * 256:(h + 1) * 256],
                                     wrcb[b][:, 2 * q:2 * q + 2, cs],
                                     xv[:, 2 * q:2 * q + 2, hs],
                                     start=False, stop=False,
                                     perf_mode=PM.DoubleRow)
                nc.tensor.matmul(pt[:, h * 256:(h + 1) * 256],
                                 wrcb[b][:, 2 * QR, cs],
                                 xv[:, 2 * QR, hs],
                                 start=False, stop=True)

        def xav(k, g):
            if g == 0:
                return x00[:] if k == 0 else x0r[:, k - 1, :]
            return xab[:, k, (g - 1) * W:g * W]

        for g in range(ng):
            if g > 0:
                # epilogue for the previous group hides under this
                # group's L1 matmuls
                emit_epilogue(4 * (g - 1), 4 * g)

            # ---- L1: [784 -> 3072], 7 fp16 passes + fp8 residual
            h1t = [ap_.tile([128, K2P, W], dt.float8e4, tag="h1a",
                            name="h1a"),
                   ap_.tile([128, K2P, W], dt.float8e4, tag="h1b",
                            name="h1b")]
            for mt in range(M1):
                pt = ps.tile([128, W], dt.float32, tag="ps")
                for k in range(K1T):
                    nc.tensor.matmul(pt[:], w1v(k, mt), xav(k, g),
                                     start=(k == 0), stop=False)
                l1_resid(pt, mt, g)
                nc.scalar.activation(h1t[mt // 12][:, mt % 12, :], pt[:],
                                     AF.Sign, bias=c1v(mt), scale=a1v(mt))

            # ---- L2-L4 iterate over 256-column halves of the group (a
            # 512-wide DoubleRow matmul paces ~0.52 ns/col vs 0.425 at 256)
            for h in range(2):
                hs = slice(h * 256, (h + 1) * 256)
                # ---- L2: [3072 -> 1536], fp8 DoubleRow
                h2t = [ap_.tile([128, K3P, 256], dt.float8e4, tag="h2a",
                                name="h2a"),
                       ap_.tile([128, K3P, 256], dt.float8e4, tag="h2b",
                                name="h2b")]
                for mt in range(M2):
                    pt = ps.tile([128, 256], dt.float32, tag="ps")
                    for kp in range(K2P):
                        nc.tensor.matmul(
                            pt[:],
                            w2sb[:, 2 * kp:2 * kp + 2,
                                 mt * 128:(mt + 1) * 128],
                            h1t[kp // 6][:, 2 * (kp % 6):2 * (kp % 6) + 2,
                                         hs],
                            start=(kp == 0), stop=(kp == K2P - 1),
                            perf_mode=PM.DoubleRow)
                    nc.scalar.activation(h2t[mt // 6][:, mt % 6, :], pt[:],
                                         AF.Sign, bias=c2v(mt),
                                         scale=a2v(mt))

                # ---- L3: [1536 -> 768], fp8 DoubleRow; scale/bias on the
                # Scalar engine (Identity), clip on DVE; bf16 output keeps
                # L4 single-pass (fp32 moving data double-pumps the PE)
                h3t = [ap_.tile([128, K4T // 2, 256], dt.bfloat16,
                                tag="h3a", name="h3a"),
                       ap_.tile([128, K4T // 2, 256], dt.bfloat16,
                                tag="h3b", name="h3b")]
                for mt in range(M3):
                    pt = ps.tile([128, 256], dt.float32, tag="ps")
                    for kp in range(K3P):
                        nc.tensor.matmul(
                            pt[:],
                            w3sb[:, 2 * kp:2 * kp + 2,
                                 mt * 128:(mt + 1) * 128],
                            h2t[kp // 3][:, 2 * (kp % 3):2 * (kp % 3) + 2,
                                         :],
                            start=(kp == 0), stop=(kp == K3P - 1),
                            perf_mode=PM.DoubleRow)
                    h3v = h3t[mt // 3][:, mt % 3, :]
                    nc.scalar.activation(h3v, pt[:], AF.Identity,
                                         bias=c3v(mt), scale=a3v(mt))
                    nc.vector.tensor_scalar(h3v, h3v, 1.0, -1.0,
                                            op0=ALU.min, op1=ALU.max)

                # ---- L4: logits z = y3 @ w4.T + b4, [batch-tile, 10]
                for bt in range(2):
                    r = 4 * g + 2 * h + bt
                    p4 = ps4.tile([128, NCLS], dt.float32, tag="p4")
                    for kt in range(K4T):
                        nc.tensor.matmul(p4[:],
                                         h3t[kt // 3][:, kt % 3,
                                             bt * 128:(bt + 1) * 128],
                                         w4sb[:, kt, :],
                                         start=(kt == 0),
                                         stop=(kt == K4T - 1))
                    nc.vector.tensor_add(zout[:, r, :], p4[:], b4v)
                    if g == ng - 1:
                        # last group: per-tile epilogue rides behind the
                        # next batch-tile's L4 matmuls
                        emit_epilogue(r, r + 1)

    nc.finalize()
    return nc


def _prep(x, w1, b1, w2, b2, w3, b3, w4, b4,
          g1, be1, m1, v1, g2, be2, m2, v2, g3, be3, m3, v3):
    """Host-side prep: transposes, binarized weight casts, BN folds, and
    the fp16 + scaled-fp8 split of x (plain transposes -- the DoubleRow
    pair row order (e p) matches the natural feature order)."""
    import concourse.mybir as mybir
    f8 = mybir.dt.np(mybir.dt.float8e4)
    bf16 = mybir.dt.np(mybir.dt.bfloat16)

    def fold(g, be, m, v, b):
        a = (g / np.sqrt(v + np.float32(BN_EPS))).astype(np.float32)
        c = (a * (b - m) + be).astype(np.float32)
        return a, c

    a1, c1 = fold(g1, be1, m1, v1, b1)
    a2, c2 = fold(g2, be2, m2, v2, b2)
    a3, c3 = fold(g3, be3, m3, v3, b3)

    def cols(v, mtiles):
        return v.reshape(mtiles, 128).T

    cstm = np.zeros((128, NCST), np.float32)
    o = 0
    for v, m in ((a1, M1), (c1, M1), (a2, M2), (c2, M2), (a3, M3), (c3, M3)):
        cstm[:, o:o + m] = cols(v, m)
        o += m
    cstm[:, o:o + NCLS] = b4.astype(np.float32)[None, :]

    s1t = np.sign(w1).T.astype(np.float32)          # [784, 3072]
    pre = dict(
        w1t=np.ascontiguousarray(s1t.astype(np.float16)),
        w1r=np.ascontiguousarray(
            (s1t * np.float32(2.0 ** -RSH)).astype(f8)),
        w2t=np.ascontiguousarray(np.sign(w2).T).astype(f8),
        w3t=np.ascontiguousarray(np.sign(w3).T).astype(f8),
        w4t=np.ascontiguousarray(w4.T).astype(bf16),
        cst=cstm,
        wrm=np.zeros((128, W), np.float16),
    )

    xa16 = x.astype(np.float16)
    r = x.astype(np.float32) - xa16.astype(np.float32)
    rb = (r * np.float32(2.0 ** RSH)).astype(f8)
    return pre, xa16.T, rb.T


def run(inputs, **spmd_kwargs):
    from concourse.bass_utils import run_bass_kernel_spmd

    if "nc" not in _cached:
        _cached["nc"] = _build(BC)
    nc = _cached["nc"]

    inputs = {k: np.asarray(v) for k, v in inputs.items()}
    pre, xat, xrt = _prep(**inputs)

    in_maps = []
    for core in range(NCORES):
        m = dict(pre)
        cs = slice(core * BC, (core + 1) * BC)
        m["xa"] = np.ascontiguousarray(xat[:, cs])
        m["xr"] = np.ascontiguousarray(xrt[:, cs])
        in_maps.append(m)

    res = run_bass_kernel_spmd(nc, in_maps, list(range(NCORES)), **spmd_kwargs)
    outs = [res.results[i]["out"] for i in range(NCORES)]
    return res, np.concatenate(outs, axis=0).astype(np.float32)


def kernel(**inputs):
    return run(inputs)[1]


# revision 19
# speedup vs baseline: 1.3629x; 1.0007x over previous
"""Trainium2 Bass kernel for a binarized 4-layer MLP (eval mode).

Reference computation (per row of x [B=16384, 784]):
  h1 = x @ sign(w1).T + b1;  s1 = sign(bn1(h1))        (clip doesn't change sign)
  h2 = s1 @ sign(w2).T + b2; s2 = sign(bn2(h2))
  h3 = s2 @ sign(w3).T + b3; y3 = clip(bn3(h3), -1, 1)
  z  = y3 @ w4.T + b4;       out = log_softmax(z)

Sharding: pure data-parallel over the batch across 8 NeuronCores
(weights replicated, no collectives).

Numerics:
  - L1 splits x on the HOST into xa = fp16(x) plus a scaled fp8 residual
    rb = fp8e4((x - xa) * 2^9).  The residual's stationary operand is
    sign(w1) * 2^-9 (exactly representable in fp8e4), so rb @ (S1*2^-9)
    accumulates the residual at its true scale into the SAME fp32 PSUM
    as the fp16 stream -- no combine step.  Total L1 error <= 2^-15|x|,
    which measures 7.6e-3 rel err end-to-end on the actual inputs
    (tolerance 2e-2; the chaotic sign-flip cascade is the amplifier).
    The fp16 stream is 7 uniform 112-row passes (784 = 7*112); the
    residual is 3 DoubleRow passes of [112, 2] row pairs (224 rows each)
    plus one plain-fp8 112-row pass -- fp8 DR does 2 rows/cycle, so the
    residual costs ~4 half-width passes instead of 7.
  - L2/L3: both operands are exactly +-1/0 in fp8e4 -> DoubleRow fp8
    matmuls produce bit-exact integer sums in fp32 PSUM.
  - BN + bias folding: bn(h + b) = A*h + C with A = g*rsqrt(v+eps),
    C = A*(b - m) + beta, applied per-partition by the Sign/Identity
    activations (fp32 internally).

Schedule notes (from NTFF trace analysis):
  - 112-row fp16 matmuls pace at the N-cycle rate (~216ns at N=512);
    128-row ones pay +40ns and 512-wide DoubleRow pays +0.1ns/col, so
    L1 uses 112-row tiles and L2/L3 stay 256 columns wide.
  - every dma_start costs ~620ns of serialized Sync-engine time to
    trigger regardless of size, so inputs/weights load as FEW large
    transfers covering all four batch groups at once; only the
    group-0 block-0 critical path keeps small fine-grained transfers.
  - a dummy-matmul burst on a zero tile warms the PE HAM clock gate
    (1.2 -> 2.4 GHz) while the startup DMAs are still in flight.
  - h1/h2/h3 activations are split into two tiles each so the next
    layer's first matmul depends only on the first half (tile-granular
    deps would stall the PE at every phase boundary).
  - the log-softmax epilogue for group g-1 hides under group g; in the
    last group it is interleaved per batch-tile behind L4.
"""

import sys

if "/opt/trn_rl_repo" not in sys.path:
    sys.path.insert(0, "/opt/trn_rl_repo")

import numpy as np

D_IN, H1, H2, H3, NCLS = 784, 3072, 1536, 768, 10
B, NCORES = 16384, 8
BC = B // NCORES          # batch rows per core
W = 512                   # batch columns per group
NG = BC // W              # groups per core
KP = 112                  # L1 k-tile partition size (784 = 7 * 112)
K1T = D_IN // KP          # 7 fp16 passes
QR = 3                    # residual DoubleRow passes ([112, 2] row pairs)
M1, M2, M3 = H1 // 128, H2 // 128, H3 // 128   # 24, 12, 6
K2P, K3P = H1 // 256, H2 // 256                # DoubleRow k-pair iters: 12, 6
K4T = H3 // 128                                # 6
NCST = 2 * M1 + 2 * M2 + 2 * M3 + NCLS         # packed per-tile consts: 94
BN_EPS = 1e-5
RSH = 9                   # residual scale: rb = fp8(r * 2^RSH), w = +-2^-RSH
NWARM = 3
NWARM2 = 10

_cached = {}


def _build(bc):
    import concourse.bacc as bacc
    import concourse.mybir as mybir
    import concourse.tile as tile

    dt = mybir.dt
    AF = mybir.ActivationFunctionType
    PM = mybir.MatmulPerfMode
    ALU = mybir.AluOpType

    ng = bc // W
    nc = bacc.Bacc("TRN2", target_bir_lowering=False, debug=False,
                   num_devices=NCORES)

    # xa = fp16(x).T per core [784, bc]; row k = feature k
    xa = nc.declare_dram_parameter("xa", [D_IN, bc], dt.float16,
                                   isOutput=False)
    # xr = fp8((x - xa) * 2^RSH).T per core [784, bc]; rows 224q..224(q+1)
    # form DoubleRow pair tile q ((e p) order), rows 672:784 the plain pass
    xr = nc.declare_dram_parameter("xr", [D_IN, bc], dt.float8e4,
                                   isOutput=False)
    w1t = nc.declare_dram_parameter("w1t", [D_IN, H1], dt.float16,
                                    isOutput=False)
    # w1r = sign(w1).T * 2^-RSH in fp8, same row scheme as xr
    w1r = nc.declare_dram_parameter("w1r", [D_IN, H1], dt.float8e4,
                                    isOutput=False)
    w2t = nc.declare_dram_parameter("w2t", [H1, H2], dt.float8e4,
                                    isOutput=False)
    w3t = nc.declare_dram_parameter("w3t", [H2, H3], dt.float8e4,
                                    isOutput=False)
    w4t = nc.declare_dram_parameter("w4t", [H3, NCLS], dt.bfloat16,
                                    isOutput=False)
    cst = nc.declare_dram_parameter("cst", [128, NCST], dt.float32,
                                    isOutput=False)
    wrm = nc.declare_dram_parameter("wrm", [128, W], dt.float16,
                                    isOutput=False)
    out = nc.declare_dram_parameter("out", [bc, NCLS], dt.float32,
                                    isOutput=True)

    with tile.TileContext(nc) as tc, \
            tc.tile_pool(name="wts", bufs=1) as wp, \
            tc.tile_pool(name="act", bufs=1) as ap_, \
            tc.tile_pool(name="eps", bufs=2) as ep, \
            tc.tile_pool(name="ps", bufs=4, space="PSUM") as ps, \
            tc.tile_pool(name="ps4", bufs=2, space="PSUM") as ps4, \
            tc.tile_pool(name="psw", bufs=1, space="PSUM") as pw:

        # ---- HAM warm-up: a short accumulate chain on a zero tile keeps
        # the PE busy while the startup DMAs stream so the clock gate
        # lifts (1.2 -> 2.4 GHz) sooner.
        wrmb = wp.tile([128, W], dt.float16, tag="wrm")
        nc.sync.dma_start(wrmb[:], wrm[:])
        pwt = pw.tile([128, W], dt.float32, tag="pw")
        for wi in range(NWARM):
            nc.tensor.matmul(pwt[:], wrmb[:, 0:128], wrmb[:],
                             start=(wi == 0), stop=(wi == NWARM - 1))

        # ---- startup-critical transfers first.  Trigger order is the
        # schedule: group-0 x and the first 512-col block of w1/w1r go
        # first (fine-grained), then one (w1, w1r) transfer per 512-col
        # block paced to the group-0 chain consumption, then the bulk.
        x00 = wp.tile([KP, W], dt.float16, tag="x00")
        nc.sync.dma_start(x00[:], xa[0:KP, 0:W])
        w1k0a = wp.tile([KP, W], dt.float16, tag="w1k0a")
        nc.sync.dma_start(w1k0a[:], w1t[0:KP, 0:W])
        x0r = wp.tile([KP, K1T - 1, W], dt.float16, tag="x0r")
        nc.sync.dma_start(x0r[:], xa[KP:D_IN, 0:W].rearrange(
            "(k p) w -> p k w", p=KP))
        w1ka = wp.tile([KP, K1T - 1, W], dt.float16, tag="w1ka")
        nc.sync.dma_start(w1ka[:], w1t[KP:D_IN, 0:W].rearrange(
            "(k p) w -> p k w", p=KP))
        cstb = wp.tile([128, NCST], dt.float32, tag="cst")
        nc.sync.dma_start(cstb[:], cst[:])
        # group-0 residual moving data: rows (2q+e)*112+p are the pair
        # tiles, rows 672:784 the plain pass -- natural feature order
        xr0 = wp.tile([KP, K1T, W], dt.float8e4, tag="xr0")
        nc.sync.dma_start(xr0[:], xr[:, 0:W].rearrange(
            "(j p) w -> p j w", p=KP))
        # w1 and residual weights per 512-col block (block 0 of w1 is the
        # two tiles above)
        wrcb = []
        t = wp.tile([KP, K1T, W], dt.float8e4, tag="wr0", name="wr0")
        nc.sync.dma_start(t[:], w1r[:, 0:W].rearrange(
            "(j p) n -> p j n", p=KP))
        wrcb.append(t)
        w1cb = [None]
        for b in range(1, M1 // 4):
            t = wp.tile([KP, K1T, W], dt.float16, tag=f"w1c{b}",
                        name=f"w1c{b}")
            nc.sync.dma_start(t[:], w1t[:, b * W:(b + 1) * W].rearrange(
                "(k p) n -> p k n", p=KP))
            w1cb.append(t)
            t = wp.tile([KP, K1T, W], dt.float8e4, tag=f"wr{b}",
                        name=f"wr{b}")
            nc.sync.dma_start(t[:], w1r[:, b * W:(b + 1) * W].rearrange(
                "(j p) n -> p j n", p=KP))
            wrcb.append(t)

        def w1v(k, mt):
            b, i = mt // 4, mt % 4
            cs = slice(i * 128, (i + 1) * 128)
            if b == 0:
                return w1k0a[:, cs] if k == 0 else w1ka[:, k - 1, cs]
            return w1cb[b][:, k, cs]

        # fp16 x and residual moving data for groups 1..3, one transfer
        xab = wp.tile([KP, K1T, (ng - 1) * W], dt.float16, tag="xab")
        nc.sync.dma_start(xab[:], xa[:, W:bc].rearrange(
            "(k p) w -> p k w", p=KP))
        xr123 = wp.tile([KP, K1T, (ng - 1) * W], dt.float8e4, tag="xr123")
        nc.sync.dma_start(xr123[:], xr[:, W:bc].rearrange(
            "(j p) w -> p j w", p=KP))

        w2sb = wp.tile([128, 2 * K2P, H2], dt.float8e4, tag="w2")
        nc.sync.dma_start(w2sb[:], w2t.ap().rearrange(
            "(t p) n -> p t n", p=128))
        w3sb = wp.tile([128, 2 * K3P, H3], dt.float8e4, tag="w3")
        nc.sync.dma_start(w3sb[:], w3t.ap().rearrange(
            "(t p) n -> p t n", p=128))
        w4sb = wp.tile([128, K4T, NCLS], dt.bfloat16, tag="w4")
        nc.sync.dma_start(w4sb[:],
                          w4t.ap().rearrange("(kt p) n -> p kt n", p=128))

        # const views into the packed per-output-tile scale/bias table
        def a1v(mt): return cstb[:, mt:mt + 1]
        def c1v(mt): return cstb[:, M1 + mt:M1 + mt + 1]
        def a2v(mt): return cstb[:, 2 * M1 + mt:2 * M1 + mt + 1]
        def c2v(mt): return cstb[:, 2 * M1 + M2 + mt:2 * M1 + M2 + mt + 1]
        def a3v(mt):
            o = 2 * M1 + 2 * M2
            return cstb[:, o + mt:o + mt + 1]
        def c3v(mt):
            o = 2 * M1 + 2 * M2 + M3
            return cstb[:, o + mt:o + mt + 1]
        b4v = cstb[:, NCST - NCLS:NCST]

        zout = wp.tile([128, ng * 4, NCLS], dt.float32, tag="zout")
        ssum = wp.tile([128, ng * 4], dt.float32, tag="ssum")
        lsum = wp.tile([128, ng * 4], dt.float32, tag="lsum")

        def emit_epilogue(lo, hi, dma=True):
            # log_softmax over the free dim; |z| is small so no max-shift
            for r in range(lo, hi):
                e = ep.tile([128, NCLS], dt.float32, tag="e")
                nc.scalar.activation(e[:], zout[:, r, :], AF.Exp,
                                     accum_out=ssum[:, r:r + 1])
            nc.scalar.activation(lsum[:, lo:hi], ssum[:, lo:hi], AF.Ln)
            for r in range(lo, hi):
                nc.vector.tensor_scalar(zout[:, r, :], zout[:, r, :],
                                        lsum[:, r:r + 1], None,
                                        op0=ALU.subtract)
            if dma:
                nc.sync.dma_start(
                    out.ap()[lo * 128:hi * 128, :].rearrange(
                        "(g p) n -> p g n", p=128),
                    zout[:, lo:hi, :])

        def l1_resid(pt, mt, g):
            # residual accumulation into pt (start=False: the fp16 stream
            # already cleared the bank); per 256-col half, 3 DoubleRow
            # pair passes + 1 plain fp8 pass
            b, i = mt // 4, mt % 4
            cs = slice(i * 128, (i + 1) * 128)
            xv = xr0 if g == 0 else xr123
            off = 0 if g == 0 else (g - 1) * W
            for h in range(2):
                hs = slice(off + h * 256, off + (h + 1) * 256)
                for q in range(QR):
                    nc.tensor.matmul(pt[:, h * 256:(h + 1) * 256],
                                     wrcb[b][:, 2 * q:2 * q + 2, cs],
                                     xv[:, 2 * q:2 * q + 2, hs],
                                     start=False, stop=False,
                                     perf_mode=PM.DoubleRow)
                nc.tensor.matmul(pt[:, h * 256:(h + 1) * 256],
                                 wrcb[b][:, 2 * QR, cs],
                                 xv[:, 2 * QR, hs],
                                 start=False, stop=True)

        def xav(k, g):
            if g == 0:
                return x00[:] if k == 0 else x0r[:, k - 1, :]
            return xab[:, k, (g - 1) * W:g * W]

        for g in range(ng):
            if g > 0:
                # epilogue for the previous group hides under this
                # group's L1 matmuls
                emit_epilogue(4 * (g - 1), 4 * g)

            # ---- L1: [784 -> 3072], 7 fp16 passes + fp8 residual
            h1t = [ap_.tile([128, K2P, W], dt.float8e4, tag="h1a",
                            name="h1a"),
                   ap_.tile([128, K2P, W], dt.float8e4, tag="h1b",
                            name="h1b")]
            for mt in range(M1):
                pt = ps.tile([128, W], dt.float32, tag="ps")
                for k in range(K1T):
                    nc.tensor.matmul(pt[:], w1v(k, mt), xav(k, g),
                                     start=(k == 0), stop=False)
                    if g == 0 and mt == 0 and k == 0:
                        # second warm burst: keeps the PE busy through the
                        # data-wait hole while the rest of the group-0
                        # transfers land, so the HAM clock gate lifts at
                        # ~13us instead of ~22us
                        for wi in range(NWARM2):
                            nc.tensor.matmul(pwt[:], wrmb[:, 0:128],
                                             wrmb[:], start=(wi == 0),
                                             stop=(wi == NWARM2 - 1))
                l1_resid(pt, mt, g)
                nc.scalar.activation(h1t[mt // 12][:, mt % 12, :], pt[:],
                                     AF.Sign, bias=c1v(mt), scale=a1v(mt))

            # ---- L2-L4 iterate over 256-column halves of the group (a
            # 512-wide DoubleRow matmul paces ~0.52 ns/col vs 0.425 at 256)
            for h in range(2):
                hs = slice(h * 256, (h + 1) * 256)
                # ---- L2: [3072 -> 1536], fp8 DoubleRow
                h2t = [ap_.tile([128, K3P, 256], dt.float8e4, tag="h2a",
                                name="h2a"),
                       ap_.tile([128, K3P, 256], dt.float8e4, tag="h2b",
                                name="h2b")]
                for mt in range(M2):
                    pt = ps.tile([128, 256], dt.float32, tag="ps")
                    for kp in range(K2P):
                        nc.tensor.matmul(
                            pt[:],
                            w2sb[:, 2 * kp:2 * kp + 2,
                                 mt * 128:(mt + 1) * 128],
                            h1t[kp // 6][:, 2 * (kp % 6):2 * (kp % 6) + 2,
                                         hs],
                            start=(kp == 0), stop=(kp == K2P - 1),
                            perf_mode=PM.DoubleRow)
                    nc.scalar.activation(h2t[mt // 6][:, mt % 6, :], pt[:],
                                         AF.Sign, bias=c2v(mt),
                                         scale=a2v(mt))

                # ---- L3: [1536 -> 768], fp8 DoubleRow; scale/bias on the
                # Scalar engine (Identity), clip on DVE; bf16 output keeps
                # L4 single-pass (fp32 moving data double-pumps the PE)
                h3t = [ap_.tile([128, K4T // 2, 256], dt.bfloat16,
                                tag="h3a", name="h3a"),
                       ap_.tile([128, K4T // 2, 256], dt.bfloat16,
                                tag="h3b", name="h3b")]
                for mt in range(M3):
                    pt = ps.tile([128, 256], dt.float32, tag="ps")
                    for kp in range(K3P):
                        nc.tensor.matmul(
                            pt[:],
                            w3sb[:, 2 * kp:2 * kp + 2,
                                 mt * 128:(mt + 1) * 128],
                            h2t[kp // 3][:, 2 * (kp % 3):2 * (kp % 3) + 2,
                                         :],
                            start=(kp == 0), stop=(kp == K3P - 1),
                            perf_mode=PM.DoubleRow)
                    h3v = h3t[mt // 3][:, mt % 3, :]
                    nc.scalar.activation(h3v, pt[:], AF.Identity,
                                         bias=c3v(mt), scale=a3v(mt))
                    nc.vector.tensor_scalar(h3v, h3v, 1.0, -1.0,
                                            op0=ALU.min, op1=ALU.max)

                # ---- L4: logits z = y3 @ w4.T + b4, [batch-tile, 10]
                for bt in range(2):
                    r = 4 * g + 2 * h + bt
                    p4 = ps4.tile([128, NCLS], dt.float32, tag="p4")
                    for kt in range(K4T):
                        nc.tensor.matmul(p4[:],
                                         h3t[kt // 3][:, kt % 3,
                                             bt * 128:(bt + 1) * 128],
                                         w4sb[:, kt, :],
                                         start=(kt == 0),
                                         stop=(kt == K4T - 1))
                    nc.vector.tensor_add(zout[:, r, :], p4[:], b4v)
                    if g == ng - 1:
                        # last group: per-tile epilogue rides behind the
                        # next batch-tile's L4 matmuls; one batched out
                        # DMA at the end (each trigger costs ~620ns of
                        # Sync time in the tail)
                        emit_epilogue(r, r + 1, dma=False)

        nc.sync.dma_start(
            out.ap()[(ng * 4 - 4) * 128:ng * 4 * 128, :].rearrange(
                "(g p) n -> p g n", p=128),
            zout[:, ng * 4 - 4:ng * 4, :])

    nc.finalize()
    return nc


def _prep(x, w1, b1, w2, b2, w3, b3, w4, b4,
          g1, be1, m1, v1, g2, be2, m2, v2, g3, be3, m3, v3):
    """Host-side prep: transposes, binarized weight casts, BN folds, and
    the fp16 + scaled-fp8 split of x (plain transposes -- the DoubleRow
    pair row order (e p) matches the natural feature order)."""
    import concourse.mybir as mybir
    f8 = mybir.dt.np(mybir.dt.float8e4)
    bf16 = mybir.dt.np(mybir.dt.bfloat16)

    def fold(g, be, m, v, b):
        a = (g / np.sqrt(v + np.float32(BN_EPS))).astype(np.float32)
        c = (a * (b - m) + be).astype(np.float32)
        return a, c

    a1, c1 = fold(g1, be1, m1, v1, b1)
    a2, c2 = fold(g2, be2, m2, v2, b2)
    a3, c3 = fold(g3, be3, m3, v3, b3)

    def cols(v, mtiles):
        return v.reshape(mtiles, 128).T

    cstm = np.zeros((128, NCST), np.float32)
    o = 0
    for v, m in ((a1, M1), (c1, M1), (a2, M2), (c2, M2), (a3, M3), (c3, M3)):
        cstm[:, o:o + m] = cols(v, m)
        o += m
    cstm[:, o:o + NCLS] = b4.astype(np.float32)[None, :]

    s1t = np.sign(w1).T.astype(np.float32)          # [784, 3072]
    pre = dict(
        w1t=np.ascontiguousarray(s1t.astype(np.float16)),
        w1r=np.ascontiguousarray(
            (s1t * np.float32(2.0 ** -RSH)).astype(f8)),
        w2t=np.ascontiguousarray(np.sign(w2).T).astype(f8),
        w3t=np.ascontiguousarray(np.sign(w3).T).astype(f8),
        w4t=np.ascontiguousarray(w4.T).astype(bf16),
        cst=cstm,
        wrm=np.zeros((128, W), np.float16),
    )

    xa16 = x.astype(np.float16)
    r = x.astype(np.float32) - xa16.astype(np.float32)
    rb = (r * np.float32(2.0 ** RSH)).astype(f8)
    return pre, xa16.T, rb.T


def run(inputs, **spmd_kwargs):
    from concourse.bass_utils import run_bass_kernel_spmd

    if "nc" not in _cached:
        _cached["nc"] = _build(BC)
    nc = _cached["nc"]

    inputs = {k: np.asarray(v) for k, v in inputs.items()}
    pre, xat, xrt = _prep(**inputs)

    in_maps = []
    for core in range(NCORES):
        m = dict(pre)
        cs = slice(core * BC, (core + 1) * BC)
        m["xa"] = np.ascontiguousarray(xat[:, cs])
        m["xr"] = np.ascontiguousarray(xrt[:, cs])
        in_maps.append(m)

    res = run_bass_kernel_spmd(nc, in_maps, list(range(NCORES)), **spmd_kwargs)
    outs = [res.results[i]["out"] for i in range(NCORES)]
    return res, np.concatenate(outs, axis=0).astype(np.float32)


def kernel(**inputs):
    return run(inputs)[1]


# revision 20
# speedup vs baseline: 1.3648x; 1.0014x over previous
"""Trainium2 Bass kernel for a binarized 4-layer MLP (eval mode).

Reference computation (per row of x [B=16384, 784]):
  h1 = x @ sign(w1).T + b1;  s1 = sign(bn1(h1))        (clip doesn't change sign)
  h2 = s1 @ sign(w2).T + b2; s2 = sign(bn2(h2))
  h3 = s2 @ sign(w3).T + b3; y3 = clip(bn3(h3), -1, 1)
  z  = y3 @ w4.T + b4;       out = log_softmax(z)

Sharding: pure data-parallel over the batch across 8 NeuronCores
(weights replicated, no collectives).

Numerics:
  - L1 splits x on the HOST into xa = fp16(x) plus a scaled fp8 residual
    rb = fp8e4((x - xa) * 2^9).  The residual's stationary operand is
    sign(w1) * 2^-9 (exactly representable in fp8e4), so rb @ (S1*2^-9)
    accumulates the residual at its true scale into the SAME fp32 PSUM
    as the fp16 stream -- no combine step.  Total L1 error <= 2^-15|x|,
    which measures 7.6e-3 rel err end-to-end on the actual inputs
    (tolerance 2e-2; the chaotic sign-flip cascade is the amplifier).
    The fp16 stream is 7 uniform 112-row passes (784 = 7*112); the
    residual is 3 DoubleRow passes of [112, 2] row pairs (224 rows each)
    plus one plain-fp8 112-row pass -- fp8 DR does 2 rows/cycle, so the
    residual costs ~4 half-width passes instead of 7.
  - L2/L3: both operands are exactly +-1/0 in fp8e4 -> DoubleRow fp8
    matmuls produce bit-exact integer sums in fp32 PSUM.
  - BN + bias folding: bn(h + b) = A*h + C with A = g*rsqrt(v+eps),
    C = A*(b - m) + beta, applied per-partition by the Sign/Identity
    activations (fp32 internally).

Schedule notes (from NTFF trace analysis):
  - 112-row fp16 matmuls pace at the N-cycle rate (~216ns at N=512);
    128-row ones pay +40ns and 512-wide DoubleRow pays +0.1ns/col, so
    L1 uses 112-row tiles and L2/L3 stay 256 columns wide.
  - every dma_start costs ~620ns of serialized Sync-engine time to
    trigger regardless of size, so inputs/weights load as FEW large
    transfers covering all four batch groups at once; only the
    group-0 block-0 critical path keeps small fine-grained transfers.
  - a dummy-matmul burst on a zero tile warms the PE HAM clock gate
    (1.2 -> 2.4 GHz) while the startup DMAs are still in flight.
  - h1/h2/h3 activations are split into two tiles each so the next
    layer's first matmul depends only on the first half (tile-granular
    deps would stall the PE at every phase boundary).
  - the log-softmax epilogue for group g-1 hides under group g; in the
    last group it is interleaved per batch-tile behind L4.
"""

import sys

if "/opt/trn_rl_repo" not in sys.path:
    sys.path.insert(0, "/opt/trn_rl_repo")

import numpy as np

D_IN, H1, H2, H3, NCLS = 784, 3072, 1536, 768, 10
B, NCORES = 16384, 8
BC = B // NCORES          # batch rows per core
W = 512                   # batch columns per group
NG = BC // W              # groups per core
KP = 112                  # L1 k-tile partition size (784 = 7 * 112)
K1T = D_IN // KP          # 7 fp16 passes
QR = 3                    # residual DoubleRow passes ([112, 2] row pairs)
M1, M2, M3 = H1 // 128, H2 // 128, H3 // 128   # 24, 12, 6
K2P, K3P = H1 // 256, H2 // 256                # DoubleRow k-pair iters: 12, 6
K4T = H3 // 128                                # 6
NCST = 2 * M1 + 2 * M2 + 2 * M3 + NCLS         # packed per-tile consts: 94
BN_EPS = 1e-5
RSH = 9                   # residual scale: rb = fp8(r * 2^RSH), w = +-2^-RSH
NWARM = 3
NWARM2 = 20

_cached = {}


def _build(bc):
    import concourse.bacc as bacc
    import concourse.mybir as mybir
    import concourse.tile as tile

    dt = mybir.dt
    AF = mybir.ActivationFunctionType
    PM = mybir.MatmulPerfMode
    ALU = mybir.AluOpType

    ng = bc // W
    nc = bacc.Bacc("TRN2", target_bir_lowering=False, debug=False,
                   num_devices=NCORES)

    # xa = fp16(x).T per core [784, bc]; row k = feature k
    xa = nc.declare_dram_parameter("xa", [D_IN, bc], dt.float16,
                                   isOutput=False)
    # xr = fp8((x - xa) * 2^RSH).T per core [784, bc]; rows 224q..224(q+1)
    # form DoubleRow pair tile q ((e p) order), rows 672:784 the plain pass
    xr = nc.declare_dram_parameter("xr", [D_IN, bc], dt.float8e4,
                                   isOutput=False)
    w1t = nc.declare_dram_parameter("w1t", [D_IN, H1], dt.float16,
                                    isOutput=False)
    # w1r = sign(w1).T * 2^-RSH in fp8, same row scheme as xr
    w1r = nc.declare_dram_parameter("w1r", [D_IN, H1], dt.float8e4,
                                    isOutput=False)
    w2t = nc.declare_dram_parameter("w2t", [H1, H2], dt.float8e4,
                                    isOutput=False)
    w3t = nc.declare_dram_parameter("w3t", [H2, H3], dt.float8e4,
                                    isOutput=False)
    w4t = nc.declare_dram_parameter("w4t", [H3, NCLS], dt.bfloat16,
                                    isOutput=False)
    cst = nc.declare_dram_parameter("cst", [128, NCST], dt.float32,
                                    isOutput=False)
    wrm = nc.declare_dram_parameter("wrm", [128, W], dt.float16,
                                    isOutput=False)
    out = nc.declare_dram_parameter("out", [bc, NCLS], dt.float32,
                                    isOutput=True)

    with tile.TileContext(nc) as tc, \
            tc.tile_pool(name="wts", bufs=1) as wp, \
            tc.tile_pool(name="act", bufs=1) as ap_, \
            tc.tile_pool(name="eps", bufs=2) as ep, \
            tc.tile_pool(name="ps", bufs=4, space="PSUM") as ps, \
            tc.tile_pool(name="ps4", bufs=2, space="PSUM") as ps4, \
            tc.tile_pool(name="psw", bufs=1, space="PSUM") as pw:

        # ---- HAM warm-up: a short accumulate chain on a zero tile keeps
        # the PE busy while the startup DMAs stream so the clock gate
        # lifts (1.2 -> 2.4 GHz) sooner.
        wrmb = wp.tile([128, W], dt.float16, tag="wrm")
        nc.sync.dma_start(wrmb[:], wrm[:])
        pwt = pw.tile([128, W], dt.float32, tag="pw")
        for wi in range(NWARM):
            nc.tensor.matmul(pwt[:], wrmb[:, 0:128], wrmb[:],
                             start=(wi == 0), stop=(wi == NWARM - 1))

        # ---- startup-critical transfers first.  Trigger order is the
        # schedule: group-0 x and the first 512-col block of w1/w1r go
        # first (fine-grained), then one (w1, w1r) transfer per 512-col
        # block paced to the group-0 chain consumption, then the bulk.
        x00 = wp.tile([KP, W], dt.float16, tag="x00")
        nc.sync.dma_start(x00[:], xa[0:KP, 0:W])
        w1k0a = wp.tile([KP, W], dt.float16, tag="w1k0a")
        nc.sync.dma_start(w1k0a[:], w1t[0:KP, 0:W])
        x0r = wp.tile([KP, K1T - 1, W], dt.float16, tag="x0r")
        nc.sync.dma_start(x0r[:], xa[KP:D_IN, 0:W].rearrange(
            "(k p) w -> p k w", p=KP))
        w1ka = wp.tile([KP, K1T - 1, W], dt.float16, tag="w1ka")
        nc.sync.dma_start(w1ka[:], w1t[KP:D_IN, 0:W].rearrange(
            "(k p) w -> p k w", p=KP))
        cstb = wp.tile([128, NCST], dt.float32, tag="cst")
        nc.sync.dma_start(cstb[:], cst[:])
        # group-0 residual moving data: rows (2q+e)*112+p are the pair
        # tiles, rows 672:784 the plain pass -- natural feature order
        xr0 = wp.tile([KP, K1T, W], dt.float8e4, tag="xr0")
        nc.sync.dma_start(xr0[:], xr[:, 0:W].rearrange(
            "(j p) w -> p j w", p=KP))
        # w1 and residual weights per 512-col block (block 0 of w1 is the
        # two tiles above)
        wrcb = []
        t = wp.tile([KP, K1T, W], dt.float8e4, tag="wr0", name="wr0")
        nc.sync.dma_start(t[:], w1r[:, 0:W].rearrange(
            "(j p) n -> p j n", p=KP))
        wrcb.append(t)
        w1cb = [None]
        for b in range(1, M1 // 4):
            t = wp.tile([KP, K1T, W], dt.float16, tag=f"w1c{b}",
                        name=f"w1c{b}")
            nc.sync.dma_start(t[:], w1t[:, b * W:(b + 1) * W].rearrange(
                "(k p) n -> p k n", p=KP))
            w1cb.append(t)
            t = wp.tile([KP, K1T, W], dt.float8e4, tag=f"wr{b}",
                        name=f"wr{b}")
            nc.sync.dma_start(t[:], w1r[:, b * W:(b + 1) * W].rearrange(
                "(j p) n -> p j n", p=KP))
            wrcb.append(t)

        def w1v(k, mt):
            b, i = mt // 4, mt % 4
            cs = slice(i * 128, (i + 1) * 128)
            if b == 0:
                return w1k0a[:, cs] if k == 0 else w1ka[:, k - 1, cs]
            return w1cb[b][:, k, cs]

        # fp16 x and residual moving data for groups 1..3, one transfer
        xab = wp.tile([KP, K1T, (ng - 1) * W], dt.float16, tag="xab")
        nc.sync.dma_start(xab[:], xa[:, W:bc].rearrange(
            "(k p) w -> p k w", p=KP))
        xr123 = wp.tile([KP, K1T, (ng - 1) * W], dt.float8e4, tag="xr123")
        nc.sync.dma_start(xr123[:], xr[:, W:bc].rearrange(
            "(j p) w -> p j w", p=KP))

        w2sb = wp.tile([128, 2 * K2P, H2], dt.float8e4, tag="w2")
        nc.sync.dma_start(w2sb[:], w2t.ap().rearrange(
            "(t p) n -> p t n", p=128))
        w3sb = wp.tile([128, 2 * K3P, H3], dt.float8e4, tag="w3")
        nc.sync.dma_start(w3sb[:], w3t.ap().rearrange(
            "(t p) n -> p t n", p=128))
        w4sb = wp.tile([128, K4T, NCLS], dt.bfloat16, tag="w4")
        nc.sync.dma_start(w4sb[:],
                          w4t.ap().rearrange("(kt p) n -> p kt n", p=128))

        # const views into the packed per-output-tile scale/bias table
        def a1v(mt): return cstb[:, mt:mt + 1]
        def c1v(mt): return cstb[:, M1 + mt:M1 + mt + 1]
        def a2v(mt): return cstb[:, 2 * M1 + mt:2 * M1 + mt + 1]
        def c2v(mt): return cstb[:, 2 * M1 + M2 + mt:2 * M1 + M2 + mt + 1]
        def a3v(mt):
            o = 2 * M1 + 2 * M2
            return cstb[:, o + mt:o + mt + 1]
        def c3v(mt):
            o = 2 * M1 + 2 * M2 + M3
            return cstb[:, o + mt:o + mt + 1]
        b4v = cstb[:, NCST - NCLS:NCST]

        zout = wp.tile([128, ng * 4, NCLS], dt.float32, tag="zout")
        ssum = wp.tile([128, ng * 4], dt.float32, tag="ssum")
        lsum = wp.tile([128, ng * 4], dt.float32, tag="lsum")

        def emit_epilogue(lo, hi, dma=True):
            # log_softmax over the free dim; |z| is small so no max-shift
            for r in range(lo, hi):
                e = ep.tile([128, NCLS], dt.float32, tag="e")
                nc.scalar.activation(e[:], zout[:, r, :], AF.Exp,
                                     accum_out=ssum[:, r:r + 1])
            nc.scalar.activation(lsum[:, lo:hi], ssum[:, lo:hi], AF.Ln)
            for r in range(lo, hi):
                nc.vector.tensor_scalar(zout[:, r, :], zout[:, r, :],
                                        lsum[:, r:r + 1], None,
                                        op0=ALU.subtract)
            if dma:
                nc.sync.dma_start(
                    out.ap()[lo * 128:hi * 128, :].rearrange(
                        "(g p) n -> p g n", p=128),
                    zout[:, lo:hi, :])

        def l1_resid(pt, mt, g):
            # residual accumulation into pt (start=False: the fp16 stream
            # already cleared the bank); per 256-col half, 3 DoubleRow
            # pair passes + 1 plain fp8 pass
            b, i = mt // 4, mt % 4
            cs = slice(i * 128, (i + 1) * 128)
            xv = xr0 if g == 0 else xr123
            off = 0 if g == 0 else (g - 1) * W
            for h in range(2):
                hs = slice(off + h * 256, off + (h + 1) * 256)
                for q in range(QR):
                    nc.tensor.matmul(pt[:, h * 256:(h + 1) * 256],
                                     wrcb[b][:, 2 * q:2 * q + 2, cs],
                                     xv[:, 2 * q:2 * q + 2, hs],
                                     start=False, stop=False,
                                     perf_mode=PM.DoubleRow)
                nc.tensor.matmul(pt[:, h * 256:(h + 1) * 256],
                                 wrcb[b][:, 2 * QR, cs],
                                 xv[:, 2 * QR, hs],
                                 start=False, stop=True)

        def xav(k, g):
            if g == 0:
                return x00[:] if k == 0 else x0r[:, k - 1, :]
            return xab[:, k, (g - 1) * W:g * W]

        for g in range(ng):
            if g > 0:
                # epilogue for the previous group hides under this
                # group's L1 matmuls
                emit_epilogue(4 * (g - 1), 4 * g)

            # ---- L1: [784 -> 3072], 7 fp16 passes + fp8 residual
            h1t = [ap_.tile([128, K2P, W], dt.float8e4, tag="h1a",
                            name="h1a"),
                   ap_.tile([128, K2P, W], dt.float8e4, tag="h1b",
                            name="h1b")]
            for mt in range(M1):
                pt = ps.tile([128, W], dt.float32, tag="ps")
                for k in range(K1T):
                    nc.tensor.matmul(pt[:], w1v(k, mt), xav(k, g),
                                     start=(k == 0), stop=False)
                    if g == 0 and mt == 0 and k == 0:
                        # second warm burst: keeps the PE busy through the
                        # data-wait hole while the rest of the group-0
                        # transfers land, so the HAM clock gate lifts at
                        # ~13us instead of ~22us
                        for wi in range(NWARM2):
                            nc.tensor.matmul(pwt[:], wrmb[:, 0:128],
                                             wrmb[:], start=(wi == 0),
                                             stop=(wi == NWARM2 - 1))
                l1_resid(pt, mt, g)
                nc.scalar.activation(h1t[mt // 12][:, mt % 12, :], pt[:],
                                     AF.Sign, bias=c1v(mt), scale=a1v(mt))

            # ---- L2-L4 iterate over 256-column halves of the group (a
            # 512-wide DoubleRow matmul paces ~0.52 ns/col vs 0.425 at 256)
            for h in range(2):
                hs = slice(h * 256, (h + 1) * 256)
                # ---- L2: [3072 -> 1536], fp8 DoubleRow
                h2t = [ap_.tile([128, K3P, 256], dt.float8e4, tag="h2a",
                                name="h2a"),
                       ap_.tile([128, K3P, 256], dt.float8e4, tag="h2b",
                                name="h2b")]
                for mt in range(M2):
                    pt = ps.tile([128, 256], dt.float32, tag="ps")
                    for kp in range(K2P):
                        nc.tensor.matmul(
                            pt[:],
                            w2sb[:, 2 * kp:2 * kp + 2,
                                 mt * 128:(mt + 1) * 128],
                            h1t[kp // 6][:, 2 * (kp % 6):2 * (kp % 6) + 2,
                                         hs],
                            start=(kp == 0), stop=(kp == K2P - 1),
                            perf_mode=PM.DoubleRow)
                    nc.scalar.activation(h2t[mt // 6][:, mt % 6, :], pt[:],
                                         AF.Sign, bias=c2v(mt),
                                         scale=a2v(mt))

                # ---- L3: [1536 -> 768], fp8 DoubleRow; scale/bias on the
                # Scalar engine (Identity), clip on DVE; bf16 output keeps
                # L4 single-pass (fp32 moving data double-pumps the PE)
                h3t = [ap_.tile([128, K4T // 2, 256], dt.bfloat16,
                                tag="h3a", name="h3a"),
                       ap_.tile([128, K4T // 2, 256], dt.bfloat16,
                                tag="h3b", name="h3b")]
                for mt in range(M3):
                    pt = ps.tile([128, 256], dt.float32, tag="ps")
                    for kp in range(K3P):
                        nc.tensor.matmul(
                            pt[:],
                            w3sb[:, 2 * kp:2 * kp + 2,
                                 mt * 128:(mt + 1) * 128],
                            h2t[kp // 3][:, 2 * (kp % 3):2 * (kp % 3) + 2,
                                         :],
                            start=(kp == 0), stop=(kp == K3P - 1),
                            perf_mode=PM.DoubleRow)
                    h3v = h3t[mt // 3][:, mt % 3, :]
                    nc.scalar.activation(h3v, pt[:], AF.Identity,
                                         bias=c3v(mt), scale=a3v(mt))
                    nc.vector.tensor_scalar(h3v, h3v, 1.0, -1.0,
                                            op0=ALU.min, op1=ALU.max)

                # ---- L4: logits z = y3 @ w4.T + b4, [batch-tile, 10]
                for bt in range(2):
                    r = 4 * g + 2 * h + bt
                    p4 = ps4.tile([128, NCLS], dt.float32, tag="p4")
                    for kt in range(K4T):
                        nc.tensor.matmul(p4[:],
                                         h3t[kt // 3][:, kt % 3,
                                             bt * 128:(bt + 1) * 128],
                                         w4sb[:, kt, :],
                                         start=(kt == 0),
                                         stop=(kt == K4T - 1))
                    nc.vector.tensor_add(zout[:, r, :], p4[:], b4v)
                    if g == ng - 1:
                        # last group: per-tile epilogue rides behind the
                        # next batch-tile's L4 matmuls; one batched out
                        # DMA at the end (each trigger costs ~620ns of
                        # Sync time in the tail)
                        emit_epilogue(r, r + 1, dma=False)

        nc.sync.dma_start(
            out.ap()[(ng * 4 - 4) * 128:ng * 4 * 128, :].rearrange(
                "(g p) n -> p g n", p=128),
            zout[:, ng * 4 - 4:ng * 4, :])

    nc.finalize()
    return nc


def _prep(x, w1, b1, w2, b2, w3, b3, w4, b4,
          g1, be1, m1, v1, g2, be2, m2, v2, g3, be3, m3, v3):
    """Host-side prep: transposes, binarized weight casts, BN folds, and
    the fp16 + scaled-fp8 split of x (plain transposes -- the DoubleRow
    pair row order (e p) matches the natural feature order)."""
    import concourse.mybir as mybir
    f8 = mybir.dt.np(mybir.dt.float8e4)
    bf16 = mybir.dt.np(mybir.dt.bfloat16)

    def fold(g, be, m, v, b):
        a = (g / np.sqrt(v + np.float32(BN_EPS))).astype(np.float32)
        c = (a * (b - m) + be).astype(np.float32)
        return a, c

    a1, c1 = fold(g1, be1, m1, v1, b1)
    a2, c2 = fold(g2, be2, m2, v2, b2)
    a3, c3 = fold(g3, be3, m3, v3, b3)

    def cols(v, mtiles):
        return v.reshape(mtiles, 128).T

    cstm = np.zeros((128, NCST), np.float32)
    o = 0
    for v, m in ((a1, M1), (c1, M1), (a2, M2), (c2, M2), (a3, M3), (c3, M3)):
        cstm[:, o:o + m] = cols(v, m)
        o += m
    cstm[:, o:o + NCLS] = b4.astype(np.float32)[None, :]

    s1t = np.sign(w1).T.astype(np.float32)          # [784, 3072]
    pre = dict(
        w1t=np.ascontiguousarray(s1t.astype(np.float16)),
        w1r=np.ascontiguousarray(
            (s1t * np.float32(2.0 ** -RSH)).astype(f8)),
        w2t=np.ascontiguousarray(np.sign(w2).T).astype(f8),
        w3t=np.ascontiguousarray(np.sign(w3).T).astype(f8),
        w4t=np.ascontiguousarray(w4.T).astype(bf16),
        cst=cstm,
        wrm=np.zeros((128, W), np.float16),
    )

    xa16 = x.astype(np.float16)
    r = x.astype(np.float32) - xa16.astype(np.float32)
    rb = (r * np.float32(2.0 ** RSH)).astype(f8)
    return pre, xa16.T, rb.T


def run(inputs, **spmd_kwargs):
    from concourse.bass_utils import run_bass_kernel_spmd

    if "nc" not in _cached:
        _cached["nc"] = _build(BC)
    nc = _cached["nc"]

    inputs = {k: np.asarray(v) for k, v in inputs.items()}
    pre, xat, xrt = _prep(**inputs)

    in_maps = []
    for core in range(NCORES):
        m = dict(pre)
        cs = slice(core * BC, (core + 1) * BC)
        m["xa"] = np.ascontiguousarray(xat[:, cs])
        m["xr"] = np.ascontiguousarray(xrt[:, cs])
        in_maps.append(m)

    res = run_bass_kernel_spmd(nc, in_maps, list(range(NCORES)), **spmd_kwargs)
    outs = [res.results[i]["out"] for i in range(NCORES)]
    return res, np.concatenate(outs, axis=0).astype(np.float32)


def kernel(**inputs):
    return run(inputs)[1]
